# revision 19
# baseline (speedup 1.0000x reference)
"""Trainium2 8-core attention kernel for nn_Attention_14104672600564.

Problem: x[4,128,64,64] f32; wq/wk/wv/wo [128,128]; bo[128].
  per (b,h): sim = (wq x)^T (wk x) * d^-.5 ; attn = softmax(sim) ; out_h = attn @ (wv x)^T
  out = wo @ concat_h(out_h^T) + bo

Sharding: 16 independent (batch, head) attention problems -> 8 cores; each core
gets one batch and one head-pair; the host unshards by summing the two cores of
each batch.

v3 design (v1: ACT-exp-bound ~284us; v2: flat pipeline + DVE exp split ~250us):
  - exp split between ACT (table Exp, bf16 pt) and DVE (custom polynomial ops;
    valid because |sim| < ~0.45 by construction). A_ACT of every 32 j-strips
    go to ACT, the rest to DVE.
  - DVE strips write exp(sim)-1 in fp8e4 (EXPM1_POLY4_ANT) into paired pt
    tiles; their attn@V matmuls run as fp8 DoubleRow pairs at 2x PE rate
    (validated on HW: one DR matmul [64,512] costs the same as one bf16
    [128,512] but covers two strips; no tile-mode drain when interleaved).
    expm1 in fp8 loses nothing: |expm1| ~ 0.1 so the absolute quantization
    error is ~30x smaller than quantizing exp itself (measured end-to-end
    rel err 3.9e-3 vs 4.0e-3 for the bf16 baseline).
  - The missing "+1" of every DVE strip is restored with a per-head column
    correction corr[p] = sum over DVE strips of colsum(fp8(V-block)) (row 0:
    the strip count, for the denominator), computed once on the PE via
    DoubleRow colsum matmuls and folded into the epilogue PSUM->SBUF
    evacuation as a per-partition bias (zero extra instructions).
  - Flat software pipeline over all 256 (h,ip,js) slots: the PE never idles
    at group boundaries and holds its full p-state. AV work trails on a
    global queue (~AVLAG slots); group epilogues fire when their last AV
    pops; the final projection po borrows a sim-pool slot PO_DELAY slots
    later so its avn dependency chain is complete when the PE reaches it.
  - GPSIMD owns all SBUF memsets, the fp8 V-block copies, and the per-chunk
    softmax normalize mul; reciprocal_approx_fast on DVE (denominators are
    ~4096, 51-ULP is plenty).
"""

import sys

sys.path.insert(0, "/opt/trn_rl_repo")

import numpy as np
import ml_dtypes

import concourse.bass as bass
import concourse.bacc as bacc
import concourse.tile as tile
from concourse import mybir
import concourse.bass_utils as _bass_utils
from concourse.bass_utils import run_bass_kernel_spmd

# ---- custom DVE exp ops (degree-4 polynomial, |x| <= ~0.5) -----------------
import concourse.dve_ops as _dvo
from concourse.dve_ops import DveOp as _DveOp
from concourse.dve_spec import (
    Spec as _Spec,
    Src0 as _Src0,
    C0 as _C0,
    C1 as _C1,
    C2 as _C2,
    One as _One,
    lower as _dve_lower,
)
from concourse.dve_uop import DveOpSpec as _DveOpSpec

# fit of exp(x) on [-0.45, 0.45] with c0=c1=1 fixed:
# exp(x) ~ 1 + x*(1 + x*(c2 + x*(c3 + x*c4))); max rel err 3.5e-5
_EXPC2, _EXPC3, _EXPC4 = 0.50019703, 0.16796468, 0.04051121


def _register_op(name, body, reference):
    for op in _dvo.OPS:
        if op.name == name:
            return op
    spec = _Spec(body=body, reference=reference)
    row = _dvo._CUSTOM_DVE_ROW_BASE + len(_dvo.OPS)
    shas = {}
    for ver in ("v3", "v4"):
        uops = _dve_lower(spec, ver=ver)
        shas[ver] = _DveOpSpec(
            name=name, opcode=row, uops=uops, rd1_en=False
        ).sha(ver)
    op = _DveOp(name, spec, subdim=False, uops_sha=shas)
    _dvo.OPS.append(op)
    _dvo._SUB_OPCODE_FOR_NAME[name] = row
    return op


def _register_exp_ops():
    t = _Src0 * _C2 + _C1
    t = t * _Src0 + _C0
    t = t * _Src0 + _One
    exp_op = _register_op(
        "EXP_POLY4_ANT",
        t * _Src0 + _One,
        lambda in0, in1, s0, s1, imm2: (
            (((imm2 * in0 + s1) * in0 + s0) * in0 + 1.0) * in0 + 1.0
        ),
    )
    em1_op = _register_op(
        "EXPM1_POLY4_ANT",
        t * _Src0,
        lambda in0, in1, s0, s1, imm2: (
            (((imm2 * in0 + s1) * in0 + s0) * in0 + 1.0) * in0
        ),
    )
    return exp_op, em1_op


_EXP_OP, _EXPM1_OP = _register_exp_ops()


BF16 = mybir.dt.bfloat16
F32 = mybir.dt.float32
FP8 = mybir.dt.float8e4
DRMODE = mybir.MatmulPerfMode.DoubleRow

HEADS = 4
DH = 32  # dim per head
C = 128  # channels
N = 4096  # tokens (64*64)
IC = 512  # i-chunk
NICH = N // IC  # 8
JS = 128  # j-strip
NJS = N // JS  # 32
VBLK = 2 * (DH + 1)  # 66: [1|Vh0|1|Vh1] per j-strip

A_ACT = 18  # j-strips per 32 whose exp runs on ACT (rest on DVE)
DVE_PER = NJS - A_ACT
CONV_PER = 8  # ACT strips per 32 whose bf16 exp GPSIMD converts to fp8 expm1
AVLAG = 8  # AV queue depth (entries) behind the sim/exp stream
PTBUFS = 13  # bf16 pt pool depth
PTPBUFS = 8  # fp8 pair-pt pool depth
PO_DELAY = 6  # slots between epilogue emission and its po matmuls

_last_results = None  # test harness pokes this for exec_time_ns / profile


def _use_dve(js):
    return ((js + 1) * DVE_PER) // NJS > (js * DVE_PER) // NJS


_DVE_ORDER = [js for js in range(NJS) if _use_dve(js)]
assert not _use_dve(0)  # js0 must be ACT: its bf16 AV carries start=True
# CONV strips: ACT-exp strips whose pt GPSIMD converts to fp8 expm1 so their
# AVs also run DoubleRow. Spread over the ACT set, never js0.
_ACT_ORDER = [js for js in range(NJS) if not _use_dve(js)]
_CONV_ORDER = [_ACT_ORDER[i] for i in range(1, 2 * CONV_PER, 2)]
_DSET = sorted(_DVE_ORDER + _CONV_ORDER)  # DoubleRow strips, paired in order
NPAIR = len(_DSET) // 2
assert len(_DSET) % 2 == 0 and 0 not in _DSET


def _ap3(base2d, d1, n1, d2, n2):
    return bass.AP(
        tensor=base2d.tensor,
        offset=base2d.offset,
        ap=[list(base2d.ap[0]), [d1, n1], [d2, n2]],
    )


def _skip_ones_ap(base2d):
    """[P, 2, 32] AP over cols {1..32, 34..65} of a [P, >=66] slice starting
    at col 1 — the V-block minus the two ones columns."""
    return _ap3(base2d, 33, 2, 1, 32)


def _build():
    nc = bacc.Bacc(None, target_bir_lowering=False)
    xt_d = nc.declare_dram_parameter("xt", [C, N], BF16, isOutput=False)
    wqkT_d = nc.declare_dram_parameter("wqkT", [C, 512], BF16, isOutput=False)
    wvT_d = nc.declare_dram_parameter("wvT", [C, VBLK], BF16, isOutput=False)
    woT_d = nc.declare_dram_parameter("woT", [C, 256], BF16, isOutput=False)
    out_d = nc.declare_dram_parameter("out", [C, N], F32, isOutput=True)
    recd = nc.dram_tensor("recd", [1, 1024], F32)  # reciprocal bounce for bcast

    EXP = mybir.ActivationFunctionType.Exp
    IDENT = mybir.ActivationFunctionType.Identity

    with tile.TileContext(nc) as tc:
        with (
            tc.tile_pool(name="singles", bufs=1) as singles,
            tc.tile_pool(name="pts", bufs=PTBUFS) as pts,
            tc.tile_pool(name="ptps", bufs=PTPBUFS) as ptps,
            tc.tile_pool(name="simpool", bufs=3, space="PSUM") as simpool,
            tc.tile_pool(name="avpool", bufs=1, space="PSUM") as avpool,
        ):
            xt_s = singles.tile([C, N], BF16)
            wqkT_s = singles.tile([C, 512], BF16)
            wvT_s = singles.tile([C, VBLK], BF16)
            woT_s = singles.tile([C, 256], BF16)
            qt0 = singles.tile([C, N], BF16)  # head0 Q: rows 0-31, rest zero
            kt0 = singles.tile([C, N], BF16)  # head0 K
            qt1 = singles.tile([C, N], BF16)  # head1 Q
            kt1 = singles.tile([C, N], BF16)  # head1 K
            # one vt tile per j-strip: [1|Vh0|1|Vh1|zeros] padded to 161 cols
            # so both heads' 128-wide lhsT APs stay in untiled (128,128) mode.
            vts = [
                singles.tile([C, 161], BF16, tag=f"vt{j}", name=f"vt{j}")
                for j in range(NJS)
            ]
            # fp8 paired V-blocks for DoubleRow: per (pair, head) a [128, 128]
            # tile, halves = [1|V(32)|0(31)] of the pair's two strips.
            vtp = [
                [
                    singles.tile([C, 128], FP8, tag=f"vtp{p}h{h}", name=f"vtp{p}h{h}")
                    for h in range(2)
                ]
                for p in range(NPAIR)
            ]
            onesb = singles.tile([C, 1], BF16)  # colsum rhs
            corrS = singles.tile([C, 2], F32)  # per-head expm1 correction col
            # per-group scratch: [a-chunk | b-chunk] halves, reused group to
            # group (WAR deps keep it safe; groups are far apart in time)
            avu = singles.tile([DH + 1, 1024], F32)
            avn = singles.tile([C, 2 * N], BF16)  # rows 33-127 zero
            rec = singles.tile([1, 1024], F32)  # 1/denom
            bc = singles.tile([DH + 1, 1024], F32)  # broadcast recips
            outs = singles.tile([C, 1024], F32)  # po evacuation

            nc.sync.dma_start(out=wqkT_s[:], in_=wqkT_d[:])
            for icd in range(NICH):
                nc.sync.dma_start(
                    out=xt_s[:, IC * icd : IC * (icd + 1)],
                    in_=xt_d[:, IC * icd : IC * (icd + 1)],
                )
            nc.gpsimd.dma_start(out=wvT_s[:], in_=wvT_d[:])
            nc.gpsimd.dma_start(out=woT_s[:], in_=woT_d[:])
            # GPSIMD prologue: only avn's junk rows (they feed the final
            # projection's contraction and must not be NaN; rows 0-32 are
            # fully written by the normalize muls) and the colsum rhs. The
            # ones columns of vts/vtp trickle on GPSIMD during early slots;
            # no zero-fills anywhere else: all junk regions (vts/vtp pad
            # columns) only ever produce PSUM rows that are never read.
            # engine APs starting at partition>0 may touch at most 32
            # partitions; zero avn's junk rows in 32-partition chunks
            # (row 32 overlaps the mul-written region: harmless WAW edge).
            nc.gpsimd.memset(avn[32:64, :], 0.0)
            nc.gpsimd.memset(onesb[:], 1.0)

            qts = [qt0, qt1]
            kts = [kt0, kt1]

            def qk_proj_one(h, icx, copy_q, copy_k):
                ps = simpool.tile([128, 1024], F32, tag="sim")
                for half in range(2):  # 0: Q, 1: K
                    cc = 2 * half + h
                    nc.tensor.matmul(
                        ps[:, IC * half : IC * (half + 1)],
                        lhsT=wqkT_s[:, 128 * cc : 128 * (cc + 1)],
                        rhs=xt_s[:, IC * icx : IC * (icx + 1)],
                        start=True,
                        stop=True,
                    )
                for half, eng, dst in ((0, copy_q, qts[h]), (1, copy_k, kts[h])):
                    eng(
                        dst[:, IC * icx : IC * (icx + 1)],
                        ps[:, IC * half : IC * (half + 1)],
                    )

            # q-copies on ACT, k-copies on DVE: halves the prologue copy
            # serialization so the first sims start ~5us sooner
            for icd in range(NICH):
                qk_proj_one(0, icd, nc.scalar.copy, nc.vector.tensor_copy)

            def v_proj_one(jc):
                pv = simpool.tile([128, 1024], F32, tag="sim")
                nc.tensor.matmul(
                    pv[:, 0:VBLK],
                    lhsT=xt_s[:, JS * jc : JS * (jc + 1)],
                    rhs=wvT_s[:],
                    start=True,
                    stop=True,
                )
                nc.scalar.copy(
                    _skip_ones_ap(vts[jc][:, 1:2]), _skip_ones_ap(pv[:, 1:2])
                )
                # fp8 paired copies for DoubleRow strips (bf16 SBUF -> fp8,
                # on GPSIMD so neither exp engine pays for it)
                if jc in _DSET:
                    k = _DSET.index(jc)
                    p, halfi = k // 2, k % 2
                    for h in range(2):
                        nc.gpsimd.tensor_copy(
                            vtp[p][h][:, 64 * halfi + 1 : 64 * halfi + 33],
                            vts[jc][:, 33 * h + 1 : 33 * h + 33],
                        )

            VLEAD = 6
            for jc in range(VLEAD):
                v_proj_one(jc)

            groups = [(h, ip) for h in range(2) for ip in range(4)]
            gstate = [dict(ptp={}, npop=0) for _ in groups]
            av_q = []  # ('b', pt, js, gidx) | ('dr', ptp_tile, pair, gidx)
            po_q = []  # (due_slot, gidx)
            ENTRIES = (NJS - len(_DSET)) + NPAIR  # av_q entries per group

            def emit_corr():
                # corr[m] = sum over DR strips of colsum of the EXACT (bf16)
                # V-block, per head. Using exact V here (not the fp8 copy the
                # DR matmuls see) makes the fp8 V quantization error multiply
                # expm1 (~0.1) instead of attn (~1.0) — a 10x suppression.
                # One 1-row matmul per (DR strip, head), stationary = the same
                # padded vts block the bf16 AVs use, so tile mode is (128,128).
                cp = simpool.tile([128, 1024], F32, tag="sim", name="corrP")
                for h in range(2):
                    for k, jc in enumerate(_DSET):
                        nc.tensor.matmul(
                            cp[:, h : h + 1],
                            lhsT=vts[jc][:, 33 * h : 33 * h + 128],
                            rhs=onesb[:],
                            start=(k == 0),
                            stop=(k == 2 * NPAIR - 1),
                            skip_group_check=True,
                        )
                nc.vector.tensor_copy(corrS[:], cp[:, 0:2])

            def emit_av(entry):
                kind, gidx = entry[0], entry[-1]
                st = gstate[gidx]
                h, ip = groups[gidx]
                if st["npop"] == 0:
                    st["ava"] = avpool.tile(
                        [C, IC], F32, tag="av_a", name=f"ava{gidx}"
                    )
                    st["avb"] = avpool.tile(
                        [C, IC], F32, tag="av_b", name=f"avb{gidx}"
                    )
                st["npop"] += 1
                first = st["npop"] == 1
                last = st["npop"] == ENTRIES
                if kind == "b":
                    _, apt, ajs, _ = entry
                    for t, av in enumerate((st["ava"], st["avb"])):
                        nc.tensor.matmul(
                            av[:],
                            lhsT=vts[ajs][:, 33 * h : 33 * h + 128],
                            rhs=apt[:, IC * t : IC * (t + 1)],
                            start=first,
                            stop=last,
                            skip_group_check=True,
                        )
                else:
                    _, ptile, pair, _ = entry
                    lhs3 = _ap3(vtp[pair][h][:, 0:1], 64, 2, 1, 64)
                    for t, av in enumerate((st["ava"], st["avb"])):
                        nc.tensor.matmul(
                            av[0:64, :],
                            lhsT=lhs3,
                            rhs=_ap3(ptile[:, IC * t : IC * t + 1], 1024, 2, 1, 512),
                            start=False,
                            stop=last,
                            perf_mode=DRMODE,
                            skip_group_check=True,
                        )
                return last

            def emit_epilogue(gidx, slot):
                h, ip = groups[gidx]
                st = gstate[gidx]
                hoff = N * h
                for t, av in enumerate((st["ava"], st["avb"])):
                    half = slice(512 * t, 512 * (t + 1))
                    # evacuate + add the expm1 correction column as bias;
                    # a-half on ACT, b-half on DVE
                    if t == 0:
                        nc.scalar.activation(
                            avu[:, half],
                            av[0 : DH + 1, :],
                            IDENT,
                            bias=corrS[0 : DH + 1, h : h + 1],
                        )
                    else:
                        nc.vector.tensor_scalar_add(
                            avu[:, half],
                            av[0 : DH + 1, :],
                            corrS[0 : DH + 1, h : h + 1],
                        )
                    nc.vector.reciprocal_approx_fast(
                        rec[0:1, half], avu[0:1, half]
                    )
                # one bounce round-trip for both chunks (fewer serial DMAs)
                nc.sync.dma_start(out=recd[0:1, 0:1024], in_=rec[0:1, 0:1024])
                dsl = recd[0:1, 0:1024]
                nc.sync.dma_start(
                    out=bc[:, 0:1024],
                    in_=bass.AP(
                        tensor=dsl.tensor,
                        offset=dsl.offset,
                        ap=[[0, DH + 1]] + list(dsl.ap[1:]),
                    ),
                )
                # normalize muls in parallel: a on DVE, b on GPSIMD
                for t in range(2):
                    ic = 2 * ip + t
                    sl = slice(hoff + IC * ic, hoff + IC * (ic + 1))
                    half = slice(512 * t, 512 * (t + 1))
                    nc.vector.tensor_mul(
                        avn[0 : DH + 1, sl], avu[:, half], bc[:, half]
                    )
                if h == 1:
                    po_q.append((slot + PO_DELAY, gidx))

            def emit_po(gidx):
                h, ip = groups[gidx]
                po = simpool.tile([128, 1024], F32, tag="sim", name=f"po{gidx}")
                for t in range(2):
                    ic = 2 * ip + t
                    nc.tensor.matmul(
                        po[:, 512 * t : 512 * t + IC],
                        lhsT=woT_s[:, 0:128],
                        rhs=avn[:, IC * ic : IC * (ic + 1)],
                        start=True,
                        stop=False,
                    )
                    nc.tensor.matmul(
                        po[:, 512 * t : 512 * t + IC],
                        lhsT=woT_s[:, 128:256],
                        rhs=avn[:, N + IC * ic : N + IC * (ic + 1)],
                        start=False,
                        stop=True,
                    )
                # bias is folded into the projection (avn row 0 = denom*recip
                # = 1, woT row 0 of block 0 is bo); evacuate the two chunks on
                # different engines so they run in parallel
                for t in range(2):
                    ic = 2 * ip + t
                    half = slice(512 * t, 512 * (t + 1))
                    eng = nc.scalar.copy if t == 0 else nc.vector.tensor_copy
                    eng(outs[:, half], po[:, half])
                    nc.sync.dma_start(
                        out=out_d[:, IC * ic : IC * (ic + 1)],
                        in_=outs[:, half],
                    )

            slot = 0
            for gidx, (h, ip) in enumerate(groups):
                qt, kt = qts[h], kts[h]
                ica, icb = 2 * ip, 2 * ip + 1
                for js in range(NJS):
                    while po_q and po_q[0][0] <= slot:
                        emit_po(po_q.pop(0)[1])
                    if gidx == 0:
                        nc.gpsimd.memset(
                            _ap3(vts[js][:, 0:1], 33, 2, 1, 1), 1.0
                        )
                        if js < NPAIR:
                            for hh in range(2):
                                nc.gpsimd.memset(
                                    _ap3(vtp[js][hh][:, 0:1], 64, 2, 1, 1), 1.0
                                )
                    if h == 0 and ip == 0 and js < NJS - VLEAD:
                        v_proj_one(js + VLEAD)
                    if h == 0 and ip == 1 and js == 0 and NPAIR > 0:
                        emit_corr()
                    if h == 0 and ip == 1 and js in (2, 4):
                        nc.vector.memset(avn[64 + 32 * (js - 2) // 2 : 96 + 32 * (js - 2) // 2, :], 0.0)
                    if h == 0 and ip in (1, 2) and js % 8 == 0:
                        qk_proj_one(
                            1,
                            4 * (ip - 1) + js // 8,
                            nc.scalar.copy,
                            nc.vector.tensor_copy,
                        )
                    sim = simpool.tile([128, 1024], F32, tag="sim")
                    for t, icx in enumerate((ica, icb)):
                        nc.tensor.matmul(
                            sim[:, IC * t : IC * (t + 1)],
                            lhsT=kt[:, JS * js : JS * (js + 1)],
                            rhs=qt[:, IC * icx : IC * (icx + 1)],
                            start=True,
                            stop=True,
                        )
                    if js in _DSET:
                        k = _DSET.index(js)
                        pair, halfi = k // 2, k % 2
                        st = gstate[gidx]
                        if halfi == 0:
                            st["ptp"][pair] = ptps.tile(
                                [128, 2048], FP8, tag="ptp",
                                name=f"ptp{gidx}_{pair}",
                            )
                        ptile = st["ptp"][pair]
                        dst = ptile[:, 1024 * halfi : 1024 * (halfi + 1)]
                        if _use_dve(js):
                            nc.vector._custom_dve(
                                _EXPM1_OP, out=dst, in0=sim[:],
                                s0=_EXPC2, s1=_EXPC3, imm2=_EXPC4,
                            )
                        else:
                            # ACT exp in bf16, GPSIMD turns it into fp8 expm1
                            pt = pts.tile([128, 1024], BF16, tag="pt")
                            nc.scalar.activation(pt[:], sim[:], EXP)
                            nc.gpsimd.tensor_scalar_add(dst, pt[:], -1.0)
                        if halfi == 1:
                            av_q.append(("dr", ptile, pair, gidx))
                    else:
                        pt = pts.tile([128, 1024], BF16, tag="pt")
                        nc.scalar.activation(pt[:], sim[:], EXP)
                        av_q.append(("b", pt, js, gidx))
                    if len(av_q) > AVLAG:
                        entry = av_q.pop(0)
                        if emit_av(entry):
                            emit_epilogue(entry[-1], slot)
                    slot += 1

            while av_q:
                entry = av_q.pop(0)
                if emit_av(entry):
                    emit_epilogue(entry[-1], slot)
                slot += 1
            while po_q:
                emit_po(po_q.pop(0)[1])
    nc.finalize()
    return nc


_nc_cache = None


def _get_nc():
    global _nc_cache
    if _nc_cache is None:
        _nc_cache = _build()
    return _nc_cache


def make_in_maps(x, wq, wk, wv, wo, bo):
    b = 4
    xt = np.asarray(x, np.float32).reshape(b, C, N)
    wq = np.asarray(wq, np.float32)
    wk = np.asarray(wk, np.float32)
    wv = np.asarray(wv, np.float32)
    wo = np.asarray(wo, np.float32)
    bo = np.asarray(bo, np.float32)
    scale = DH ** (-0.5)

    def bf(a):
        return np.ascontiguousarray(a.astype(ml_dtypes.bfloat16))

    in_maps = []
    for core in range(8):
        bi, hp = core // 2, core % 2
        wq2 = wq[64 * hp : 64 * hp + 64] * scale
        wk2 = wk[64 * hp : 64 * hp + 64]
        wv2 = wv[64 * hp : 64 * hp + 64]
        wqkT = np.zeros((C, 512), np.float32)
        wqkT[:, 0:32] = wq2.T[:, 0:32]  # Qh0
        wqkT[:, 128:160] = wq2.T[:, 32:64]  # Qh1
        wqkT[:, 256:288] = wk2.T[:, 0:32]  # Kh0
        wqkT[:, 384:416] = wk2.T[:, 32:64]  # Kh1
        wvT = np.zeros((C, VBLK), np.float32)  # cols 0,33 stay 0 (ones in SBUF)
        wvT[:, 1:33] = wv2.T[:, 0:32]
        wvT[:, 34:66] = wv2.T[:, 32:64]
        woT = np.zeros((C, 256), np.float32)
        woT[1:33, 0:128] = wo[:, 64 * hp : 64 * hp + 32].T
        woT[1:33, 128:256] = wo[:, 64 * hp + 32 : 64 * hp + 64].T
        if hp == 0:
            woT[0, 0:128] = bo  # bias rides avn row 0 (= denom/denom = 1)
        in_maps.append(
            {
                "xt": bf(xt[bi]),
                "wqkT": bf(wqkT),
                "wvT": bf(wvT),
                "woT": bf(woT),
            }
        )
    return in_maps


def kernel(x, wq, wk, wv, wo, bo):
    global _last_results
    in_maps = make_in_maps(x, wq, wk, wv, wo, bo)
    nc = _get_nc()
    res = run_bass_kernel_spmd(nc, in_maps, core_ids=list(range(8)))
    _last_results = res
    outs = res.results
    out = np.zeros((4, C, N), np.float32)
    for bi in range(4):
        out[bi] = np.asarray(outs[2 * bi]["out"], np.float32) + np.asarray(
            outs[2 * bi + 1]["out"], np.float32
        )
    return out.reshape(4, C, 64, 64)


# revision 20
# speedup vs baseline: 4.0819x; 4.0819x over previous
"""Trainium2 8-core attention kernel for nn_Attention_14104672600564.

Problem: x[4,128,64,64] f32; wq/wk/wv/wo [128,128]; bo[128].
  per (b,h): sim = (wq x)^T (wk x) * d^-.5 ; attn = softmax(sim) ; out_h = attn @ (wv x)^T
  out = wo @ concat_h(out_h^T) + bo

Sharding: 16 independent (batch, head) attention problems -> 8 cores; each core
gets one batch and one head-pair; the host unshards by summing the two cores of
each batch.

v3 design (v1: ACT-exp-bound ~284us; v2: flat pipeline + DVE exp split ~250us):
  - exp split between ACT (table Exp, bf16 pt) and DVE (custom polynomial ops;
    valid because |sim| < ~0.45 by construction). A_ACT of every 32 j-strips
    go to ACT, the rest to DVE.
  - DVE strips write exp(sim)-1 in fp8e4 (EXPM1_POLY4_ANT) into paired pt
    tiles; their attn@V matmuls run as fp8 DoubleRow pairs at 2x PE rate
    (validated on HW: one DR matmul [64,512] costs the same as one bf16
    [128,512] but covers two strips; no tile-mode drain when interleaved).
    expm1 in fp8 loses nothing: |expm1| ~ 0.1 so the absolute quantization
    error is ~30x smaller than quantizing exp itself (measured end-to-end
    rel err 3.9e-3 vs 4.0e-3 for the bf16 baseline).
  - The missing "+1" of every DVE strip is restored with a per-head column
    correction corr[p] = sum over DVE strips of colsum(fp8(V-block)) (row 0:
    the strip count, for the denominator), computed once on the PE via
    DoubleRow colsum matmuls and folded into the epilogue PSUM->SBUF
    evacuation as a per-partition bias (zero extra instructions).
  - Flat software pipeline over all 256 (h,ip,js) slots: the PE never idles
    at group boundaries and holds its full p-state. AV work trails on a
    global queue (~AVLAG slots); group epilogues fire when their last AV
    pops; the final projection po borrows a sim-pool slot PO_DELAY slots
    later so its avn dependency chain is complete when the PE reaches it.
  - GPSIMD owns all SBUF memsets, the fp8 V-block copies, and the per-chunk
    softmax normalize mul; reciprocal_approx_fast on DVE (denominators are
    ~4096, 51-ULP is plenty).
"""

import sys

sys.path.insert(0, "/opt/trn_rl_repo")

import numpy as np
import ml_dtypes

import concourse.bass as bass
import concourse.bacc as bacc
import concourse.tile as tile
from concourse import mybir
import concourse.bass_utils as _bass_utils
from concourse.bass_utils import run_bass_kernel_spmd

# ---- custom DVE exp ops (degree-4 polynomial, |x| <= ~0.5) -----------------
import concourse.dve_ops as _dvo
from concourse.dve_ops import DveOp as _DveOp
from concourse.dve_spec import (
    Spec as _Spec,
    Src0 as _Src0,
    C0 as _C0,
    C1 as _C1,
    C2 as _C2,
    One as _One,
    lower as _dve_lower,
)
from concourse.dve_uop import DveOpSpec as _DveOpSpec

# fit of exp(x) on [-0.45, 0.45] with c0=c1=1 fixed:
# exp(x) ~ 1 + x*(1 + x*(c2 + x*(c3 + x*c4))); max rel err 3.5e-5
_EXPC2, _EXPC3, _EXPC4 = 0.50019703, 0.16796468, 0.04051121


def _register_op(name, body, reference):
    for op in _dvo.OPS:
        if op.name == name:
            return op
    spec = _Spec(body=body, reference=reference)
    row = _dvo._CUSTOM_DVE_ROW_BASE + len(_dvo.OPS)
    shas = {}
    for ver in ("v3", "v4"):
        uops = _dve_lower(spec, ver=ver)
        shas[ver] = _DveOpSpec(
            name=name, opcode=row, uops=uops, rd1_en=False
        ).sha(ver)
    op = _DveOp(name, spec, subdim=False, uops_sha=shas)
    _dvo.OPS.append(op)
    _dvo._SUB_OPCODE_FOR_NAME[name] = row
    return op


def _register_exp_ops():
    t = _Src0 * _C2 + _C1
    t = t * _Src0 + _C0
    t = t * _Src0 + _One
    exp_op = _register_op(
        "EXP_POLY4_ANT",
        t * _Src0 + _One,
        lambda in0, in1, s0, s1, imm2: (
            (((imm2 * in0 + s1) * in0 + s0) * in0 + 1.0) * in0 + 1.0
        ),
    )
    em1_op = _register_op(
        "EXPM1_POLY4_ANT",
        t * _Src0,
        lambda in0, in1, s0, s1, imm2: (
            (((imm2 * in0 + s1) * in0 + s0) * in0 + 1.0) * in0
        ),
    )
    return exp_op, em1_op


_EXP_OP, _EXPM1_OP = _register_exp_ops()


BF16 = mybir.dt.bfloat16
F32 = mybir.dt.float32
FP8 = mybir.dt.float8e4
DRMODE = mybir.MatmulPerfMode.DoubleRow

HEADS = 4
DH = 32  # dim per head
C = 128  # channels
N = 4096  # tokens (64*64)
IC = 512  # i-chunk
NICH = N // IC  # 8
JS = 128  # j-strip
NJS = N // JS  # 32
VBLK = 2 * (DH + 1)  # 66: [1|Vh0|1|Vh1] per j-strip

A_ACT = 18  # j-strips per 32 whose exp runs on ACT (rest on DVE)
DVE_PER = NJS - A_ACT
CONV_PER = 0  # (GPSIMD fp8 conversion measured 14.7us/strip: unusable)
AVLAG = 8  # AV queue depth (entries) behind the sim/exp stream
PTBUFS = 13  # bf16 pt pool depth
PTPBUFS = 8  # fp8 pair-pt pool depth
PO_DELAY = 6  # slots between epilogue emission and its po matmuls

_last_results = None  # test harness pokes this for exec_time_ns / profile


def _use_dve(js):
    return ((js + 1) * DVE_PER) // NJS > (js * DVE_PER) // NJS


_DVE_ORDER = [js for js in range(NJS) if _use_dve(js)]
assert not _use_dve(0)  # js0 must be ACT: its bf16 AV carries start=True
# CONV strips: ACT-exp strips whose pt GPSIMD converts to fp8 expm1 so their
# AVs also run DoubleRow. Spread over the ACT set, never js0.
_ACT_ORDER = [js for js in range(NJS) if not _use_dve(js)]
_CONV_ORDER = [_ACT_ORDER[i] for i in range(1, 2 * CONV_PER, 2)]
_DSET = sorted(_DVE_ORDER + _CONV_ORDER)  # DoubleRow strips, paired in order
NPAIR = len(_DSET) // 2
assert len(_DSET) % 2 == 0 and 0 not in _DSET


def _ap3(base2d, d1, n1, d2, n2):
    return bass.AP(
        tensor=base2d.tensor,
        offset=base2d.offset,
        ap=[list(base2d.ap[0]), [d1, n1], [d2, n2]],
    )


def _skip_ones_ap(base2d):
    """[P, 2, 32] AP over cols {1..32, 34..65} of a [P, >=66] slice starting
    at col 1 — the V-block minus the two ones columns."""
    return _ap3(base2d, 33, 2, 1, 32)


def _build():
    nc = bacc.Bacc(None, target_bir_lowering=False)
    xt_d = nc.declare_dram_parameter("xt", [C, N], BF16, isOutput=False)
    wqkT_d = nc.declare_dram_parameter("wqkT", [C, 512], BF16, isOutput=False)
    wvT_d = nc.declare_dram_parameter("wvT", [C, VBLK], BF16, isOutput=False)
    woT_d = nc.declare_dram_parameter("woT", [C, 256], BF16, isOutput=False)
    out_d = nc.declare_dram_parameter("out", [C, N], F32, isOutput=True)
    recd = nc.dram_tensor("recd", [1, 1024], F32)  # reciprocal bounce for bcast

    EXP = mybir.ActivationFunctionType.Exp
    IDENT = mybir.ActivationFunctionType.Identity

    with tile.TileContext(nc) as tc:
        with (
            tc.tile_pool(name="singles", bufs=1) as singles,
            tc.tile_pool(name="pts", bufs=PTBUFS) as pts,
            tc.tile_pool(name="ptps", bufs=PTPBUFS) as ptps,
            tc.tile_pool(name="simpool", bufs=3, space="PSUM") as simpool,
            tc.tile_pool(name="avpool", bufs=1, space="PSUM") as avpool,
        ):
            xt_s = singles.tile([C, N], BF16)
            wqkT_s = singles.tile([C, 512], BF16)
            wvT_s = singles.tile([C, VBLK], BF16)
            woT_s = singles.tile([C, 256], BF16)
            qt0 = singles.tile([C, N], BF16)  # head0 Q: rows 0-31, rest zero
            kt0 = singles.tile([C, N], BF16)  # head0 K
            qt1 = singles.tile([C, N], BF16)  # head1 Q
            kt1 = singles.tile([C, N], BF16)  # head1 K
            # one vt tile per j-strip: [1|Vh0|1|Vh1|zeros] padded to 161 cols
            # so both heads' 128-wide lhsT APs stay in untiled (128,128) mode.
            vts = [
                singles.tile([C, 161], BF16, tag=f"vt{j}", name=f"vt{j}")
                for j in range(NJS)
            ]
            # fp8 paired V-blocks for DoubleRow: per (pair, head) a [128, 128]
            # tile, halves = [1|V(32)|0(31)] of the pair's two strips.
            vtp = [
                [
                    singles.tile([C, 128], FP8, tag=f"vtp{p}h{h}", name=f"vtp{p}h{h}")
                    for h in range(2)
                ]
                for p in range(NPAIR)
            ]
            onesb = singles.tile([C, 1], BF16)  # colsum rhs
            corrS = singles.tile([C, 2], F32)  # per-head expm1 correction col
            # per-group scratch: [a-chunk | b-chunk] halves, reused group to
            # group (WAR deps keep it safe; groups are far apart in time)
            avu = singles.tile([DH + 1, 1024], F32)
            avn = singles.tile([C, 2 * N], BF16)  # rows 33-127 zero
            rec = singles.tile([1, 1024], F32)  # 1/denom
            bc = singles.tile([DH + 1, 1024], F32)  # broadcast recips
            outs = singles.tile([C, 1024], F32)  # po evacuation

            nc.sync.dma_start(out=wqkT_s[:], in_=wqkT_d[:])
            for icd in range(NICH):
                nc.sync.dma_start(
                    out=xt_s[:, IC * icd : IC * (icd + 1)],
                    in_=xt_d[:, IC * icd : IC * (icd + 1)],
                )
            nc.gpsimd.dma_start(out=wvT_s[:], in_=wvT_d[:])
            nc.gpsimd.dma_start(out=woT_s[:], in_=woT_d[:])
            # GPSIMD prologue: only avn's junk rows (they feed the final
            # projection's contraction and must not be NaN; rows 0-32 are
            # fully written by the normalize muls) and the colsum rhs. The
            # ones columns of vts/vtp trickle on GPSIMD during early slots;
            # no zero-fills anywhere else: all junk regions (vts/vtp pad
            # columns) only ever produce PSUM rows that are never read.
            # engine APs starting at partition>0 may touch at most 32
            # partitions; zero avn's junk rows in 32-partition chunks
            # (row 32 overlaps the mul-written region: harmless WAW edge).
            nc.gpsimd.memset(avn[32:64, :], 0.0)
            nc.gpsimd.memset(onesb[:], 1.0)

            qts = [qt0, qt1]
            kts = [kt0, kt1]

            def qk_proj_one(h, icx, copy_q, copy_k):
                ps = simpool.tile([128, 1024], F32, tag="sim")
                for half in range(2):  # 0: Q, 1: K
                    cc = 2 * half + h
                    nc.tensor.matmul(
                        ps[:, IC * half : IC * (half + 1)],
                        lhsT=wqkT_s[:, 128 * cc : 128 * (cc + 1)],
                        rhs=xt_s[:, IC * icx : IC * (icx + 1)],
                        start=True,
                        stop=True,
                    )
                for half, eng, dst in ((0, copy_q, qts[h]), (1, copy_k, kts[h])):
                    eng(
                        dst[:, IC * icx : IC * (icx + 1)],
                        ps[:, IC * half : IC * (half + 1)],
                    )

            # q-copies on ACT, k-copies on DVE: halves the prologue copy
            # serialization so the first sims start ~5us sooner
            for icd in range(NICH):
                qk_proj_one(0, icd, nc.scalar.copy, nc.vector.tensor_copy)

            def v_proj_one(jc):
                pv = simpool.tile([128, 1024], F32, tag="sim")
                nc.tensor.matmul(
                    pv[:, 0:VBLK],
                    lhsT=xt_s[:, JS * jc : JS * (jc + 1)],
                    rhs=wvT_s[:],
                    start=True,
                    stop=True,
                )
                nc.scalar.copy(
                    _skip_ones_ap(vts[jc][:, 1:2]), _skip_ones_ap(pv[:, 1:2])
                )
                # fp8 paired copies for DoubleRow strips (bf16 SBUF -> fp8,
                # on GPSIMD so neither exp engine pays for it)
                if jc in _DSET:
                    k = _DSET.index(jc)
                    p, halfi = k // 2, k % 2
                    for h in range(2):
                        nc.gpsimd.tensor_copy(
                            vtp[p][h][:, 64 * halfi + 1 : 64 * halfi + 33],
                            vts[jc][:, 33 * h + 1 : 33 * h + 33],
                        )

            VLEAD = 6
            for jc in range(VLEAD):
                v_proj_one(jc)

            groups = [(h, ip) for h in range(2) for ip in range(4)]
            gstate = [dict(ptp={}, npop=0) for _ in groups]
            av_q = []  # ('b', pt, js, gidx) | ('dr', ptp_tile, pair, gidx)
            po_q = []  # (due_slot, gidx)
            ENTRIES = (NJS - len(_DSET)) + NPAIR  # av_q entries per group

            def emit_corr():
                # corr[m] = sum over DR strips of colsum of the EXACT (bf16)
                # V-block, per head. Using exact V here (not the fp8 copy the
                # DR matmuls see) makes the fp8 V quantization error multiply
                # expm1 (~0.1) instead of attn (~1.0) — a 10x suppression.
                # One 1-row matmul per (DR strip, head), stationary = the same
                # padded vts block the bf16 AVs use, so tile mode is (128,128).
                cp = simpool.tile([128, 1024], F32, tag="sim", name="corrP")
                for h in range(2):
                    for k, jc in enumerate(_DSET):
                        nc.tensor.matmul(
                            cp[:, h : h + 1],
                            lhsT=vts[jc][:, 33 * h : 33 * h + 128],
                            rhs=onesb[:],
                            start=(k == 0),
                            stop=(k == 2 * NPAIR - 1),
                            skip_group_check=True,
                        )
                nc.vector.tensor_copy(corrS[:], cp[:, 0:2])

            def emit_av(entry):
                kind, gidx = entry[0], entry[-1]
                st = gstate[gidx]
                h, ip = groups[gidx]
                if st["npop"] == 0:
                    st["ava"] = avpool.tile(
                        [C, IC], F32, tag="av_a", name=f"ava{gidx}"
                    )
                    st["avb"] = avpool.tile(
                        [C, IC], F32, tag="av_b", name=f"avb{gidx}"
                    )
                st["npop"] += 1
                first = st["npop"] == 1
                last = st["npop"] == ENTRIES
                if kind == "b":
                    _, apt, ajs, _ = entry
                    for t, av in enumerate((st["ava"], st["avb"])):
                        nc.tensor.matmul(
                            av[:],
                            lhsT=vts[ajs][:, 33 * h : 33 * h + 128],
                            rhs=apt[:, IC * t : IC * (t + 1)],
                            start=first,
                            stop=last,
                            skip_group_check=True,
                        )
                else:
                    _, ptile, pair, _ = entry
                    lhs3 = _ap3(vtp[pair][h][:, 0:1], 64, 2, 1, 64)
                    for t, av in enumerate((st["ava"], st["avb"])):
                        nc.tensor.matmul(
                            av[0:64, :],
                            lhsT=lhs3,
                            rhs=_ap3(ptile[:, IC * t : IC * t + 1], 1024, 2, 1, 512),
                            start=False,
                            stop=last,
                            perf_mode=DRMODE,
                            skip_group_check=True,
                        )
                return last

            def emit_epilogue(gidx, slot):
                h, ip = groups[gidx]
                st = gstate[gidx]
                hoff = N * h
                for t, av in enumerate((st["ava"], st["avb"])):
                    half = slice(512 * t, 512 * (t + 1))
                    # evacuate + add the expm1 correction column as bias;
                    # a-half on ACT, b-half on DVE
                    if t == 0:
                        nc.scalar.activation(
                            avu[:, half],
                            av[0 : DH + 1, :],
                            IDENT,
                            bias=corrS[0 : DH + 1, h : h + 1],
                        )
                    else:
                        nc.vector.tensor_scalar_add(
                            avu[:, half],
                            av[0 : DH + 1, :],
                            corrS[0 : DH + 1, h : h + 1],
                        )
                    nc.vector.reciprocal_approx_fast(
                        rec[0:1, half], avu[0:1, half]
                    )
                # one bounce round-trip for both chunks (fewer serial DMAs)
                nc.sync.dma_start(out=recd[0:1, 0:1024], in_=rec[0:1, 0:1024])
                dsl = recd[0:1, 0:1024]
                nc.sync.dma_start(
                    out=bc[:, 0:1024],
                    in_=bass.AP(
                        tensor=dsl.tensor,
                        offset=dsl.offset,
                        ap=[[0, DH + 1]] + list(dsl.ap[1:]),
                    ),
                )
                # normalize muls in parallel: a on DVE, b on GPSIMD
                for t in range(2):
                    ic = 2 * ip + t
                    sl = slice(hoff + IC * ic, hoff + IC * (ic + 1))
                    half = slice(512 * t, 512 * (t + 1))
                    eng = nc.vector if t == 0 else nc.gpsimd
                    eng.tensor_mul(
                        avn[0 : DH + 1, sl], avu[:, half], bc[:, half]
                    )
                if h == 1:
                    po_q.append((slot + PO_DELAY, gidx))

            def emit_po(gidx):
                h, ip = groups[gidx]
                po = simpool.tile([128, 1024], F32, tag="sim", name=f"po{gidx}")
                for t in range(2):
                    ic = 2 * ip + t
                    nc.tensor.matmul(
                        po[:, 512 * t : 512 * t + IC],
                        lhsT=woT_s[:, 0:128],
                        rhs=avn[:, IC * ic : IC * (ic + 1)],
                        start=True,
                        stop=False,
                    )
                    nc.tensor.matmul(
                        po[:, 512 * t : 512 * t + IC],
                        lhsT=woT_s[:, 128:256],
                        rhs=avn[:, N + IC * ic : N + IC * (ic + 1)],
                        start=False,
                        stop=True,
                    )
                # bias is folded into the projection (avn row 0 = denom*recip
                # = 1, woT row 0 of block 0 is bo); evacuate the two chunks on
                # different engines so they run in parallel
                for t in range(2):
                    ic = 2 * ip + t
                    half = slice(512 * t, 512 * (t + 1))
                    eng = nc.scalar.copy if t == 0 else nc.vector.tensor_copy
                    eng(outs[:, half], po[:, half])
                    nc.sync.dma_start(
                        out=out_d[:, IC * ic : IC * (ic + 1)],
                        in_=outs[:, half],
                    )

            slot = 0
            for gidx, (h, ip) in enumerate(groups):
                qt, kt = qts[h], kts[h]
                ica, icb = 2 * ip, 2 * ip + 1
                for js in range(NJS):
                    while po_q and po_q[0][0] <= slot:
                        emit_po(po_q.pop(0)[1])
                    if gidx == 0:
                        nc.gpsimd.memset(
                            _ap3(vts[js][:, 0:1], 33, 2, 1, 1), 1.0
                        )
                        if js < NPAIR:
                            for hh in range(2):
                                nc.gpsimd.memset(
                                    _ap3(vtp[js][hh][:, 0:1], 64, 2, 1, 1), 1.0
                                )
                    if h == 0 and ip == 0 and js < NJS - VLEAD:
                        v_proj_one(js + VLEAD)
                    if h == 0 and ip == 1 and js == 0 and NPAIR > 0:
                        emit_corr()
                    if h == 0 and ip == 1 and js in (2, 4):
                        nc.vector.memset(avn[64 + 32 * (js - 2) // 2 : 96 + 32 * (js - 2) // 2, :], 0.0)
                    if h == 0 and ip in (1, 2) and js % 8 == 0:
                        qk_proj_one(
                            1,
                            4 * (ip - 1) + js // 8,
                            nc.scalar.copy,
                            nc.vector.tensor_copy,
                        )
                    sim = simpool.tile([128, 1024], F32, tag="sim")
                    for t, icx in enumerate((ica, icb)):
                        nc.tensor.matmul(
                            sim[:, IC * t : IC * (t + 1)],
                            lhsT=kt[:, JS * js : JS * (js + 1)],
                            rhs=qt[:, IC * icx : IC * (icx + 1)],
                            start=True,
                            stop=True,
                        )
                    if js in _DSET:
                        k = _DSET.index(js)
                        pair, halfi = k // 2, k % 2
                        st = gstate[gidx]
                        if halfi == 0:
                            st["ptp"][pair] = ptps.tile(
                                [128, 2048], FP8, tag="ptp",
                                name=f"ptp{gidx}_{pair}",
                            )
                        ptile = st["ptp"][pair]
                        dst = ptile[:, 1024 * halfi : 1024 * (halfi + 1)]
                        if _use_dve(js):
                            nc.vector._custom_dve(
                                _EXPM1_OP, out=dst, in0=sim[:],
                                s0=_EXPC2, s1=_EXPC3, imm2=_EXPC4,
                            )
                        else:
                            # ACT exp in bf16, GPSIMD turns it into fp8 expm1
                            pt = pts.tile([128, 1024], BF16, tag="pt")
                            nc.scalar.activation(pt[:], sim[:], EXP)
                            nc.gpsimd.tensor_scalar_add(dst, pt[:], -1.0)
                        if halfi == 1:
                            av_q.append(("dr", ptile, pair, gidx))
                    else:
                        pt = pts.tile([128, 1024], BF16, tag="pt")
                        nc.scalar.activation(pt[:], sim[:], EXP)
                        av_q.append(("b", pt, js, gidx))
                    if len(av_q) > AVLAG:
                        entry = av_q.pop(0)
                        if emit_av(entry):
                            emit_epilogue(entry[-1], slot)
                    slot += 1

            while av_q:
                entry = av_q.pop(0)
                if emit_av(entry):
                    emit_epilogue(entry[-1], slot)
                slot += 1
            while po_q:
                emit_po(po_q.pop(0)[1])
    nc.finalize()
    return nc


_nc_cache = None


def _get_nc():
    global _nc_cache
    if _nc_cache is None:
        _nc_cache = _build()
    return _nc_cache


def make_in_maps(x, wq, wk, wv, wo, bo):
    b = 4
    xt = np.asarray(x, np.float32).reshape(b, C, N)
    wq = np.asarray(wq, np.float32)
    wk = np.asarray(wk, np.float32)
    wv = np.asarray(wv, np.float32)
    wo = np.asarray(wo, np.float32)
    bo = np.asarray(bo, np.float32)
    scale = DH ** (-0.5)

    def bf(a):
        return np.ascontiguousarray(a.astype(ml_dtypes.bfloat16))

    in_maps = []
    for core in range(8):
        bi, hp = core // 2, core % 2
        wq2 = wq[64 * hp : 64 * hp + 64] * scale
        wk2 = wk[64 * hp : 64 * hp + 64]
        wv2 = wv[64 * hp : 64 * hp + 64]
        wqkT = np.zeros((C, 512), np.float32)
        wqkT[:, 0:32] = wq2.T[:, 0:32]  # Qh0
        wqkT[:, 128:160] = wq2.T[:, 32:64]  # Qh1
        wqkT[:, 256:288] = wk2.T[:, 0:32]  # Kh0
        wqkT[:, 384:416] = wk2.T[:, 32:64]  # Kh1
        wvT = np.zeros((C, VBLK), np.float32)  # cols 0,33 stay 0 (ones in SBUF)
        wvT[:, 1:33] = wv2.T[:, 0:32]
        wvT[:, 34:66] = wv2.T[:, 32:64]
        woT = np.zeros((C, 256), np.float32)
        woT[1:33, 0:128] = wo[:, 64 * hp : 64 * hp + 32].T
        woT[1:33, 128:256] = wo[:, 64 * hp + 32 : 64 * hp + 64].T
        if hp == 0:
            woT[0, 0:128] = bo  # bias rides avn row 0 (= denom/denom = 1)
        in_maps.append(
            {
                "xt": bf(xt[bi]),
                "wqkT": bf(wqkT),
                "wvT": bf(wvT),
                "woT": bf(woT),
            }
        )
    return in_maps


def kernel(x, wq, wk, wv, wo, bo):
    global _last_results
    in_maps = make_in_maps(x, wq, wk, wv, wo, bo)
    nc = _get_nc()
    res = run_bass_kernel_spmd(nc, in_maps, core_ids=list(range(8)))
    _last_results = res
    outs = res.results
    out = np.zeros((4, C, N), np.float32)
    for bi in range(4):
        out[bi] = np.asarray(outs[2 * bi]["out"], np.float32) + np.asarray(
            outs[2 * bi + 1]["out"], np.float32
        )
    return out.reshape(4, C, 64, 64)


# revision 21
# speedup vs baseline: 4.2368x; 1.0379x over previous
"""Trainium2 8-core attention kernel for nn_Attention_14104672600564.

Problem: x[4,128,64,64] f32; wq/wk/wv/wo [128,128]; bo[128].
  per (b,h): sim = (wq x)^T (wk x) * d^-.5 ; attn = softmax(sim) ; out_h = attn @ (wv x)^T
  out = wo @ concat_h(out_h^T) + bo

Sharding: 16 independent (batch, head) attention problems -> 8 cores; each core
gets one batch and one head-pair; the host unshards by summing the two cores of
each batch.

v3 design (v1: ACT-exp-bound ~284us; v2: flat pipeline + DVE exp split ~250us):
  - exp split between ACT (table Exp, bf16 pt) and DVE (custom polynomial ops;
    valid because |sim| < ~0.45 by construction). A_ACT of every 32 j-strips
    go to ACT, the rest to DVE.
  - DVE strips write exp(sim)-1 in fp8e4 (EXPM1_POLY4_ANT) into paired pt
    tiles; their attn@V matmuls run as fp8 DoubleRow pairs at 2x PE rate
    (validated on HW: one DR matmul [64,512] costs the same as one bf16
    [128,512] but covers two strips; no tile-mode drain when interleaved).
    expm1 in fp8 loses nothing: |expm1| ~ 0.1 so the absolute quantization
    error is ~30x smaller than quantizing exp itself (measured end-to-end
    rel err 3.9e-3 vs 4.0e-3 for the bf16 baseline).
  - The missing "+1" of every DVE strip is restored with a per-head column
    correction corr[p] = sum over DVE strips of colsum(fp8(V-block)) (row 0:
    the strip count, for the denominator), computed once on the PE via
    DoubleRow colsum matmuls and folded into the epilogue PSUM->SBUF
    evacuation as a per-partition bias (zero extra instructions).
  - Flat software pipeline over all 256 (h,ip,js) slots: the PE never idles
    at group boundaries and holds its full p-state. AV work trails on a
    global queue (~AVLAG slots); group epilogues fire when their last AV
    pops; the final projection po borrows a sim-pool slot PO_DELAY slots
    later so its avn dependency chain is complete when the PE reaches it.
  - GPSIMD owns all SBUF memsets, the fp8 V-block copies, and the per-chunk
    softmax normalize mul; reciprocal_approx_fast on DVE (denominators are
    ~4096, 51-ULP is plenty).
"""

import sys

sys.path.insert(0, "/opt/trn_rl_repo")

import numpy as np
import ml_dtypes

import concourse.bass as bass
import concourse.bacc as bacc
import concourse.tile as tile
from concourse import mybir
import concourse.bass_utils as _bass_utils
from concourse.bass_utils import run_bass_kernel_spmd

# ---- custom DVE exp ops (degree-4 polynomial, |x| <= ~0.5) -----------------
import concourse.dve_ops as _dvo
from concourse.dve_ops import DveOp as _DveOp
from concourse.dve_spec import (
    Spec as _Spec,
    Src0 as _Src0,
    C0 as _C0,
    C1 as _C1,
    C2 as _C2,
    One as _One,
    lower as _dve_lower,
)
from concourse.dve_uop import DveOpSpec as _DveOpSpec

# fit of exp(x) on [-0.45, 0.45] with c0=c1=1 fixed:
# exp(x) ~ 1 + x*(1 + x*(c2 + x*(c3 + x*c4))); max rel err 3.5e-5
_EXPC2, _EXPC3, _EXPC4 = 0.50019703, 0.16796468, 0.04051121


def _register_op(name, body, reference):
    for op in _dvo.OPS:
        if op.name == name:
            return op
    spec = _Spec(body=body, reference=reference)
    row = _dvo._CUSTOM_DVE_ROW_BASE + len(_dvo.OPS)
    shas = {}
    for ver in ("v3", "v4"):
        uops = _dve_lower(spec, ver=ver)
        shas[ver] = _DveOpSpec(
            name=name, opcode=row, uops=uops, rd1_en=False
        ).sha(ver)
    op = _DveOp(name, spec, subdim=False, uops_sha=shas)
    _dvo.OPS.append(op)
    _dvo._SUB_OPCODE_FOR_NAME[name] = row
    return op


def _register_exp_ops():
    t = _Src0 * _C2 + _C1
    t = t * _Src0 + _C0
    t = t * _Src0 + _One
    exp_op = _register_op(
        "EXP_POLY4_ANT",
        t * _Src0 + _One,
        lambda in0, in1, s0, s1, imm2: (
            (((imm2 * in0 + s1) * in0 + s0) * in0 + 1.0) * in0 + 1.0
        ),
    )
    em1_op = _register_op(
        "EXPM1_POLY4_ANT",
        t * _Src0,
        lambda in0, in1, s0, s1, imm2: (
            (((imm2 * in0 + s1) * in0 + s0) * in0 + 1.0) * in0
        ),
    )
    return exp_op, em1_op


_EXP_OP, _EXPM1_OP = _register_exp_ops()


BF16 = mybir.dt.bfloat16
F32 = mybir.dt.float32
FP8 = mybir.dt.float8e4
DRMODE = mybir.MatmulPerfMode.DoubleRow

HEADS = 4
DH = 32  # dim per head
C = 128  # channels
N = 4096  # tokens (64*64)
IC = 512  # i-chunk
NICH = N // IC  # 8
JS = 128  # j-strip
NJS = N // JS  # 32
VBLK = 2 * (DH + 1)  # 66: [1|Vh0|1|Vh1] per j-strip

A_ACT = 18  # j-strips per 32 whose exp runs on ACT (rest on DVE)
DVE_PER = NJS - A_ACT
CONV_PER = 0  # (GPSIMD fp8 conversion measured 14.7us/strip: unusable)
AVLAG = 8  # AV queue depth (entries) behind the sim/exp stream
PTBUFS = 13  # bf16 pt pool depth
PTPBUFS = 8  # fp8 pair-pt pool depth
PO_DELAY = 6  # slots between epilogue emission and its po matmuls

_last_results = None  # test harness pokes this for exec_time_ns / profile


def _use_dve(js):
    return ((js + 1) * DVE_PER) // NJS > (js * DVE_PER) // NJS


_DVE_ORDER = [js for js in range(NJS) if _use_dve(js)]
assert not _use_dve(0)  # js0 must be ACT: its bf16 AV carries start=True
# CONV strips: ACT-exp strips whose pt GPSIMD converts to fp8 expm1 so their
# AVs also run DoubleRow. Spread over the ACT set, never js0.
_ACT_ORDER = [js for js in range(NJS) if not _use_dve(js)]
_CONV_ORDER = [_ACT_ORDER[i] for i in range(1, 2 * CONV_PER, 2)]
_DSET = sorted(_DVE_ORDER + _CONV_ORDER)  # DoubleRow strips, paired in order
NPAIR = len(_DSET) // 2
assert len(_DSET) % 2 == 0 and 0 not in _DSET


def _ap3(base2d, d1, n1, d2, n2):
    return bass.AP(
        tensor=base2d.tensor,
        offset=base2d.offset,
        ap=[list(base2d.ap[0]), [d1, n1], [d2, n2]],
    )


def _skip_ones_ap(base2d):
    """[P, 2, 32] AP over cols {1..32, 34..65} of a [P, >=66] slice starting
    at col 1 — the V-block minus the two ones columns."""
    return _ap3(base2d, 33, 2, 1, 32)


def _build():
    nc = bacc.Bacc(None, target_bir_lowering=False)
    xt_d = nc.declare_dram_parameter("xt", [C, N], BF16, isOutput=False)
    wqkT_d = nc.declare_dram_parameter("wqkT", [C, 512], BF16, isOutput=False)
    wvT_d = nc.declare_dram_parameter("wvT", [C, VBLK], BF16, isOutput=False)
    woT_d = nc.declare_dram_parameter("woT", [C, 256], BF16, isOutput=False)
    out_d = nc.declare_dram_parameter("out", [C, N], F32, isOutput=True)
    recd = nc.dram_tensor("recd", [1, 1024], F32)  # reciprocal bounce for bcast

    EXP = mybir.ActivationFunctionType.Exp
    IDENT = mybir.ActivationFunctionType.Identity

    with tile.TileContext(nc) as tc:
        with (
            tc.tile_pool(name="singles", bufs=1) as singles,
            tc.tile_pool(name="pts", bufs=PTBUFS) as pts,
            tc.tile_pool(name="ptps", bufs=PTPBUFS) as ptps,
            tc.tile_pool(name="simpool", bufs=3, space="PSUM") as simpool,
            tc.tile_pool(name="avpool", bufs=1, space="PSUM") as avpool,
        ):
            xt_s = singles.tile([C, N], BF16)
            wqkT_s = singles.tile([C, 512], BF16)
            wvT_s = singles.tile([C, VBLK], BF16)
            woT_s = singles.tile([C, 256], BF16)
            qt0 = singles.tile([C, N], BF16)  # head0 Q: rows 0-31, rest zero
            kt0 = singles.tile([C, N], BF16)  # head0 K
            qt1 = singles.tile([C, N], BF16)  # head1 Q
            kt1 = singles.tile([C, N], BF16)  # head1 K
            # one vt tile per j-strip: [1|Vh0|1|Vh1|zeros] padded to 161 cols
            # so both heads' 128-wide lhsT APs stay in untiled (128,128) mode.
            vts = [
                singles.tile([C, 161], BF16, tag=f"vt{j}", name=f"vt{j}")
                for j in range(NJS)
            ]
            # fp8 paired V-blocks for DoubleRow: per (pair, head) a [128, 128]
            # tile, halves = [1|V(32)|0(31)] of the pair's two strips.
            vtp = [
                [
                    singles.tile([C, 128], FP8, tag=f"vtp{p}h{h}", name=f"vtp{p}h{h}")
                    for h in range(2)
                ]
                for p in range(NPAIR)
            ]
            onesb = singles.tile([C, 1], BF16)  # colsum rhs
            corrS = singles.tile([C, 2], F32)  # per-head expm1 correction col
            # per-group scratch: [a-chunk | b-chunk] halves, reused group to
            # group (WAR deps keep it safe; groups are far apart in time)
            avu = singles.tile([DH + 1, 1024], F32)
            avn = singles.tile([C, 2 * N], BF16)  # rows 33-127 zero
            rec = singles.tile([1, 1024], F32)  # 1/denom
            bc = singles.tile([DH + 1, 1024], F32)  # broadcast recips
            outs = singles.tile([C, 1024], F32)  # po evacuation

            nc.sync.dma_start(out=wqkT_s[:], in_=wqkT_d[:])
            for icd in range(NICH):
                nc.sync.dma_start(
                    out=xt_s[:, IC * icd : IC * (icd + 1)],
                    in_=xt_d[:, IC * icd : IC * (icd + 1)],
                )
            nc.gpsimd.dma_start(out=wvT_s[:], in_=wvT_d[:])
            nc.gpsimd.dma_start(out=woT_s[:], in_=woT_d[:])
            # GPSIMD prologue: only avn's junk rows (they feed the final
            # projection's contraction and must not be NaN; rows 0-32 are
            # fully written by the normalize muls) and the colsum rhs. The
            # ones columns of vts/vtp trickle on GPSIMD during early slots;
            # no zero-fills anywhere else: all junk regions (vts/vtp pad
            # columns) only ever produce PSUM rows that are never read.
            # engine APs starting at partition>0 may touch at most 32
            # partitions; zero avn's junk rows in 32-partition chunks
            # (row 32 overlaps the mul-written region: harmless WAW edge).
            nc.gpsimd.memset(avn[32:64, :], 0.0)
            nc.gpsimd.memset(avn[64:96, :], 0.0)
            nc.gpsimd.memset(avn[96:128, :], 0.0)
            nc.gpsimd.memset(onesb[:], 1.0)

            qts = [qt0, qt1]
            kts = [kt0, kt1]

            def qk_proj_one(h, icx, copy_q, copy_k):
                ps = simpool.tile([128, 1024], F32, tag="sim")
                for half in range(2):  # 0: Q, 1: K
                    cc = 2 * half + h
                    nc.tensor.matmul(
                        ps[:, IC * half : IC * (half + 1)],
                        lhsT=wqkT_s[:, 128 * cc : 128 * (cc + 1)],
                        rhs=xt_s[:, IC * icx : IC * (icx + 1)],
                        start=True,
                        stop=True,
                    )
                for half, eng, dst in ((0, copy_q, qts[h]), (1, copy_k, kts[h])):
                    eng(
                        dst[:, IC * icx : IC * (icx + 1)],
                        ps[:, IC * half : IC * (half + 1)],
                    )

            # q-copies on ACT, k-copies on DVE: halves the prologue copy
            # serialization so the first sims start ~5us sooner
            for icd in range(NICH):
                qk_proj_one(0, icd, nc.scalar.copy, nc.vector.tensor_copy)

            def v_proj_one(jc):
                pv = simpool.tile([128, 1024], F32, tag="sim")
                nc.tensor.matmul(
                    pv[:, 0:VBLK],
                    lhsT=xt_s[:, JS * jc : JS * (jc + 1)],
                    rhs=wvT_s[:],
                    start=True,
                    stop=True,
                )
                nc.scalar.copy(
                    _skip_ones_ap(vts[jc][:, 1:2]), _skip_ones_ap(pv[:, 1:2])
                )
                # fp8 paired copies for DoubleRow strips (bf16 SBUF -> fp8,
                # on GPSIMD so neither exp engine pays for it)
                if jc in _DSET:
                    k = _DSET.index(jc)
                    p, halfi = k // 2, k % 2
                    for h in range(2):
                        nc.gpsimd.tensor_copy(
                            vtp[p][h][:, 64 * halfi + 1 : 64 * halfi + 33],
                            vts[jc][:, 33 * h + 1 : 33 * h + 33],
                        )

            VLEAD = 6
            for jc in range(VLEAD):
                v_proj_one(jc)

            groups = [(h, ip) for h in range(2) for ip in range(4)]
            gstate = [dict(ptp={}, npop=0) for _ in groups]
            av_q = []  # ('b', pt, js, gidx) | ('dr', ptp_tile, pair, gidx)
            po_q = []  # (due_slot, gidx)
            ENTRIES = (NJS - len(_DSET)) + NPAIR  # av_q entries per group

            def emit_corr():
                # corr[m] = sum over DR strips of colsum of the EXACT (bf16)
                # V-block, per head. Using exact V here (not the fp8 copy the
                # DR matmuls see) makes the fp8 V quantization error multiply
                # expm1 (~0.1) instead of attn (~1.0) — a 10x suppression.
                # One 1-row matmul per (DR strip, head), stationary = the same
                # padded vts block the bf16 AVs use, so tile mode is (128,128).
                cp = simpool.tile([128, 1024], F32, tag="sim", name="corrP")
                for h in range(2):
                    for k, jc in enumerate(_DSET):
                        nc.tensor.matmul(
                            cp[:, h : h + 1],
                            lhsT=vts[jc][:, 33 * h : 33 * h + 128],
                            rhs=onesb[:],
                            start=(k == 0),
                            stop=(k == 2 * NPAIR - 1),
                            skip_group_check=True,
                        )
                nc.vector.tensor_copy(corrS[:], cp[:, 0:2])

            def emit_av(entry):
                kind, gidx = entry[0], entry[-1]
                st = gstate[gidx]
                h, ip = groups[gidx]
                if st["npop"] == 0:
                    st["ava"] = avpool.tile(
                        [C, IC], F32, tag="av_a", name=f"ava{gidx}"
                    )
                    st["avb"] = avpool.tile(
                        [C, IC], F32, tag="av_b", name=f"avb{gidx}"
                    )
                st["npop"] += 1
                first = st["npop"] == 1
                last = st["npop"] == ENTRIES
                if kind == "b":
                    _, apt, ajs, _ = entry
                    for t, av in enumerate((st["ava"], st["avb"])):
                        nc.tensor.matmul(
                            av[:],
                            lhsT=vts[ajs][:, 33 * h : 33 * h + 128],
                            rhs=apt[:, IC * t : IC * (t + 1)],
                            start=first,
                            stop=last,
                            skip_group_check=True,
                        )
                else:
                    _, ptile, pair, _ = entry
                    lhs3 = _ap3(vtp[pair][h][:, 0:1], 64, 2, 1, 64)
                    for t, av in enumerate((st["ava"], st["avb"])):
                        nc.tensor.matmul(
                            av[0:64, :],
                            lhsT=lhs3,
                            rhs=_ap3(ptile[:, IC * t : IC * t + 1], 1024, 2, 1, 512),
                            start=False,
                            stop=last,
                            perf_mode=DRMODE,
                            skip_group_check=True,
                        )
                return last

            def emit_epilogue(gidx, slot):
                h, ip = groups[gidx]
                st = gstate[gidx]
                hoff = N * h
                for t, av in enumerate((st["ava"], st["avb"])):
                    half = slice(512 * t, 512 * (t + 1))
                    # evacuate + add the expm1 correction column as bias;
                    # a-half on ACT, b-half on DVE
                    if t == 0:
                        nc.scalar.activation(
                            avu[:, half],
                            av[0 : DH + 1, :],
                            IDENT,
                            bias=corrS[0 : DH + 1, h : h + 1],
                        )
                    else:
                        nc.vector.tensor_scalar_add(
                            avu[:, half],
                            av[0 : DH + 1, :],
                            corrS[0 : DH + 1, h : h + 1],
                        )
                    nc.vector.reciprocal_approx_fast(
                        rec[0:1, half], avu[0:1, half]
                    )
                # one bounce round-trip for both chunks (fewer serial DMAs)
                nc.sync.dma_start(out=recd[0:1, 0:1024], in_=rec[0:1, 0:1024])
                dsl = recd[0:1, 0:1024]
                nc.sync.dma_start(
                    out=bc[:, 0:1024],
                    in_=bass.AP(
                        tensor=dsl.tensor,
                        offset=dsl.offset,
                        ap=[[0, DH + 1]] + list(dsl.ap[1:]),
                    ),
                )
                # normalize muls in parallel: a on DVE, b on GPSIMD
                for t in range(2):
                    ic = 2 * ip + t
                    sl = slice(hoff + IC * ic, hoff + IC * (ic + 1))
                    half = slice(512 * t, 512 * (t + 1))
                    eng = nc.vector if t == 0 else nc.gpsimd
                    eng.tensor_mul(
                        avn[0 : DH + 1, sl], avu[:, half], bc[:, half]
                    )
                if h == 1:
                    po_q.append((slot + PO_DELAY, gidx))

            def emit_po(gidx):
                h, ip = groups[gidx]
                po = simpool.tile([128, 1024], F32, tag="sim", name=f"po{gidx}")
                for t in range(2):
                    ic = 2 * ip + t
                    nc.tensor.matmul(
                        po[:, 512 * t : 512 * t + IC],
                        lhsT=woT_s[:, 0:128],
                        rhs=avn[:, IC * ic : IC * (ic + 1)],
                        start=True,
                        stop=False,
                    )
                    nc.tensor.matmul(
                        po[:, 512 * t : 512 * t + IC],
                        lhsT=woT_s[:, 128:256],
                        rhs=avn[:, N + IC * ic : N + IC * (ic + 1)],
                        start=False,
                        stop=True,
                    )
                # bias is folded into the projection (avn row 0 = denom*recip
                # = 1, woT row 0 of block 0 is bo); evacuate the two chunks on
                # different engines so they run in parallel
                for t in range(2):
                    ic = 2 * ip + t
                    half = slice(512 * t, 512 * (t + 1))
                    eng = nc.scalar.copy if t == 0 else nc.vector.tensor_copy
                    eng(outs[:, half], po[:, half])
                    nc.sync.dma_start(
                        out=out_d[:, IC * ic : IC * (ic + 1)],
                        in_=outs[:, half],
                    )

            slot = 0
            for gidx, (h, ip) in enumerate(groups):
                qt, kt = qts[h], kts[h]
                ica, icb = 2 * ip, 2 * ip + 1
                for js in range(NJS):
                    while po_q and po_q[0][0] <= slot:
                        emit_po(po_q.pop(0)[1])
                    if gidx == 0:
                        nc.gpsimd.memset(
                            _ap3(vts[js][:, 0:1], 33, 2, 1, 1), 1.0
                        )
                        if js < NPAIR:
                            for hh in range(2):
                                nc.gpsimd.memset(
                                    _ap3(vtp[js][hh][:, 0:1], 64, 2, 1, 1), 1.0
                                )
                    if h == 0 and ip == 0 and js < NJS - VLEAD:
                        v_proj_one(js + VLEAD)
                    if h == 0 and ip == 1 and js == 0 and NPAIR > 0:
                        emit_corr()
                    if h == 0 and ip in (1, 2) and js % 8 == 0:
                        qk_proj_one(
                            1,
                            4 * (ip - 1) + js // 8,
                            nc.scalar.copy,
                            nc.vector.tensor_copy,
                        )
                    sim = simpool.tile([128, 1024], F32, tag="sim")
                    for t, icx in enumerate((ica, icb)):
                        nc.tensor.matmul(
                            sim[:, IC * t : IC * (t + 1)],
                            lhsT=kt[:, JS * js : JS * (js + 1)],
                            rhs=qt[:, IC * icx : IC * (icx + 1)],
                            start=True,
                            stop=True,
                        )
                    if js in _DSET:
                        k = _DSET.index(js)
                        pair, halfi = k // 2, k % 2
                        st = gstate[gidx]
                        if halfi == 0:
                            st["ptp"][pair] = ptps.tile(
                                [128, 2048], FP8, tag="ptp",
                                name=f"ptp{gidx}_{pair}",
                            )
                        ptile = st["ptp"][pair]
                        dst = ptile[:, 1024 * halfi : 1024 * (halfi + 1)]
                        if _use_dve(js):
                            nc.vector._custom_dve(
                                _EXPM1_OP, out=dst, in0=sim[:],
                                s0=_EXPC2, s1=_EXPC3, imm2=_EXPC4,
                            )
                        else:
                            # ACT exp in bf16, GPSIMD turns it into fp8 expm1
                            pt = pts.tile([128, 1024], BF16, tag="pt")
                            nc.scalar.activation(pt[:], sim[:], EXP)
                            nc.gpsimd.tensor_scalar_add(dst, pt[:], -1.0)
                        if halfi == 1:
                            av_q.append(("dr", ptile, pair, gidx))
                    else:
                        pt = pts.tile([128, 1024], BF16, tag="pt")
                        nc.scalar.activation(pt[:], sim[:], EXP)
                        av_q.append(("b", pt, js, gidx))
                    if len(av_q) > AVLAG:
                        entry = av_q.pop(0)
                        if emit_av(entry):
                            emit_epilogue(entry[-1], slot)
                    slot += 1

            while av_q:
                entry = av_q.pop(0)
                if emit_av(entry):
                    emit_epilogue(entry[-1], slot)
                slot += 1
            while po_q:
                emit_po(po_q.pop(0)[1])
    nc.finalize()
    return nc


_nc_cache = None


def _get_nc():
    global _nc_cache
    if _nc_cache is None:
        _nc_cache = _build()
    return _nc_cache


def make_in_maps(x, wq, wk, wv, wo, bo):
    b = 4
    xt = np.asarray(x, np.float32).reshape(b, C, N)
    wq = np.asarray(wq, np.float32)
    wk = np.asarray(wk, np.float32)
    wv = np.asarray(wv, np.float32)
    wo = np.asarray(wo, np.float32)
    bo = np.asarray(bo, np.float32)
    scale = DH ** (-0.5)

    def bf(a):
        return np.ascontiguousarray(a.astype(ml_dtypes.bfloat16))

    in_maps = []
    for core in range(8):
        bi, hp = core // 2, core % 2
        wq2 = wq[64 * hp : 64 * hp + 64] * scale
        wk2 = wk[64 * hp : 64 * hp + 64]
        wv2 = wv[64 * hp : 64 * hp + 64]
        wqkT = np.zeros((C, 512), np.float32)
        wqkT[:, 0:32] = wq2.T[:, 0:32]  # Qh0
        wqkT[:, 128:160] = wq2.T[:, 32:64]  # Qh1
        wqkT[:, 256:288] = wk2.T[:, 0:32]  # Kh0
        wqkT[:, 384:416] = wk2.T[:, 32:64]  # Kh1
        wvT = np.zeros((C, VBLK), np.float32)  # cols 0,33 stay 0 (ones in SBUF)
        wvT[:, 1:33] = wv2.T[:, 0:32]
        wvT[:, 34:66] = wv2.T[:, 32:64]
        woT = np.zeros((C, 256), np.float32)
        woT[1:33, 0:128] = wo[:, 64 * hp : 64 * hp + 32].T
        woT[1:33, 128:256] = wo[:, 64 * hp + 32 : 64 * hp + 64].T
        if hp == 0:
            woT[0, 0:128] = bo  # bias rides avn row 0 (= denom/denom = 1)
        in_maps.append(
            {
                "xt": bf(xt[bi]),
                "wqkT": bf(wqkT),
                "wvT": bf(wvT),
                "woT": bf(woT),
            }
        )
    return in_maps


def kernel(x, wq, wk, wv, wo, bo):
    global _last_results
    in_maps = make_in_maps(x, wq, wk, wv, wo, bo)
    nc = _get_nc()
    res = run_bass_kernel_spmd(nc, in_maps, core_ids=list(range(8)))
    _last_results = res
    outs = res.results
    out = np.zeros((4, C, N), np.float32)
    for bi in range(4):
        out[bi] = np.asarray(outs[2 * bi]["out"], np.float32) + np.asarray(
            outs[2 * bi + 1]["out"], np.float32
        )
    return out.reshape(4, C, 64, 64)


# revision 22
# speedup vs baseline: 4.2526x; 1.0037x over previous
"""Trainium2 8-core attention kernel for nn_Attention_14104672600564.

Problem: x[4,128,64,64] f32; wq/wk/wv/wo [128,128]; bo[128].
  per (b,h): sim = (wq x)^T (wk x) * d^-.5 ; attn = softmax(sim) ; out_h = attn @ (wv x)^T
  out = wo @ concat_h(out_h^T) + bo

Sharding: 16 independent (batch, head) attention problems -> 8 cores; each core
gets one batch and one head-pair; the host unshards by summing the two cores of
each batch.

v3 design (v1: ACT-exp-bound ~284us; v2: flat pipeline + DVE exp split ~250us):
  - exp split between ACT (table Exp, bf16 pt) and DVE (custom polynomial ops;
    valid because |sim| < ~0.45 by construction). A_ACT of every 32 j-strips
    go to ACT, the rest to DVE.
  - DVE strips write exp(sim)-1 in fp8e4 (EXPM1_POLY4_ANT) into paired pt
    tiles; their attn@V matmuls run as fp8 DoubleRow pairs at 2x PE rate
    (validated on HW: one DR matmul [64,512] costs the same as one bf16
    [128,512] but covers two strips; no tile-mode drain when interleaved).
    expm1 in fp8 loses nothing: |expm1| ~ 0.1 so the absolute quantization
    error is ~30x smaller than quantizing exp itself (measured end-to-end
    rel err 3.9e-3 vs 4.0e-3 for the bf16 baseline).
  - The missing "+1" of every DVE strip is restored with a per-head column
    correction corr[m] = sum over DR strips of colsum(V-block) (row 0: the
    strip count, for the denominator), computed once on the PE from the
    EXACT bf16 V blocks (so the fp8 V quantization error multiplies expm1
    ~0.1 instead of attn ~1.0 - a 10x suppression) and folded into the
    epilogue PSUM->SBUF evacuation as a per-partition bias (zero extra
    instructions). The reciprocal reads the corrected denominator row.
  - Flat software pipeline over all 256 (h,ip,js) slots: the PE never idles
    at group boundaries and holds its full p-state. AV work trails on a
    global queue (~AVLAG slots); group epilogues fire when their last AV
    pops; the final projection po borrows a sim-pool slot PO_DELAY slots
    later so its avn dependency chain is complete when the PE reaches it.
  - GPSIMD owns the avn junk-row zeroing (the only memset that is
    correctness-required: those rows feed the final projection contraction),
    the fp8 V-block copies, the ones-column writes, and half the normalize
    muls; reciprocal_approx_fast on DVE (denominators ~4096, 51-ULP plenty).
    All other zero-fills were removed: vts/vtp pad columns only ever produce
    PSUM rows that no reader touches.
"""

import sys

sys.path.insert(0, "/opt/trn_rl_repo")

import numpy as np
import ml_dtypes

import concourse.bass as bass
import concourse.bacc as bacc
import concourse.tile as tile
from concourse import mybir
import concourse.bass_utils as _bass_utils
from concourse.bass_utils import run_bass_kernel_spmd

# ---- custom DVE exp ops (degree-4 polynomial, |x| <= ~0.5) -----------------
import concourse.dve_ops as _dvo
from concourse.dve_ops import DveOp as _DveOp
from concourse.dve_spec import (
    Spec as _Spec,
    Src0 as _Src0,
    C0 as _C0,
    C1 as _C1,
    C2 as _C2,
    One as _One,
    lower as _dve_lower,
)
from concourse.dve_uop import DveOpSpec as _DveOpSpec

# fit of exp(x) on [-0.45, 0.45] with c0=c1=1 fixed:
# exp(x) ~ 1 + x*(1 + x*(c2 + x*(c3 + x*c4))); max rel err 3.5e-5
_EXPC2, _EXPC3, _EXPC4 = 0.50019703, 0.16796468, 0.04051121


def _register_op(name, body, reference):
    for op in _dvo.OPS:
        if op.name == name:
            return op
    spec = _Spec(body=body, reference=reference)
    row = _dvo._CUSTOM_DVE_ROW_BASE + len(_dvo.OPS)
    shas = {}
    for ver in ("v3", "v4"):
        uops = _dve_lower(spec, ver=ver)
        shas[ver] = _DveOpSpec(
            name=name, opcode=row, uops=uops, rd1_en=False
        ).sha(ver)
    op = _DveOp(name, spec, subdim=False, uops_sha=shas)
    _dvo.OPS.append(op)
    _dvo._SUB_OPCODE_FOR_NAME[name] = row
    return op


def _register_exp_ops():
    t = _Src0 * _C2 + _C1
    t = t * _Src0 + _C0
    t = t * _Src0 + _One
    exp_op = _register_op(
        "EXP_POLY4_ANT",
        t * _Src0 + _One,
        lambda in0, in1, s0, s1, imm2: (
            (((imm2 * in0 + s1) * in0 + s0) * in0 + 1.0) * in0 + 1.0
        ),
    )
    em1_op = _register_op(
        "EXPM1_POLY4_ANT",
        t * _Src0,
        lambda in0, in1, s0, s1, imm2: (
            (((imm2 * in0 + s1) * in0 + s0) * in0 + 1.0) * in0
        ),
    )
    return exp_op, em1_op


_EXP_OP, _EXPM1_OP = _register_exp_ops()


BF16 = mybir.dt.bfloat16
F32 = mybir.dt.float32
FP8 = mybir.dt.float8e4
DRMODE = mybir.MatmulPerfMode.DoubleRow

HEADS = 4
DH = 32  # dim per head
C = 128  # channels
N = 4096  # tokens (64*64)
IC = 512  # i-chunk
NICH = N // IC  # 8
JS = 128  # j-strip
NJS = N // JS  # 32
VBLK = 2 * (DH + 1)  # 66: [1|Vh0|1|Vh1] per j-strip

A_ACT = 18  # j-strips per 32 whose exp runs on ACT (rest on DVE)
DVE_PER = NJS - A_ACT
CONV_PER = 0  # (GPSIMD fp8 conversion measured 14.7us/strip: unusable)
AVLAG = 8  # AV queue depth (entries) behind the sim/exp stream
PTBUFS = 13  # bf16 pt pool depth
PTPBUFS = 8  # fp8 pair-pt pool depth
PO_DELAY = 6  # slots between epilogue emission and its po matmuls

_last_results = None  # test harness pokes this for exec_time_ns / profile


def _use_dve(js):
    return ((js + 1) * DVE_PER) // NJS > (js * DVE_PER) // NJS


_DVE_ORDER = [js for js in range(NJS) if _use_dve(js)]
assert not _use_dve(0)  # js0 must be ACT: its bf16 AV carries start=True
# CONV strips: ACT-exp strips whose pt GPSIMD converts to fp8 expm1 so their
# AVs also run DoubleRow. Spread over the ACT set, never js0.
_ACT_ORDER = [js for js in range(NJS) if not _use_dve(js)]
_CONV_ORDER = [_ACT_ORDER[i] for i in range(1, 2 * CONV_PER, 2)]
_DSET = sorted(_DVE_ORDER + _CONV_ORDER)  # DoubleRow strips, paired in order
NPAIR = len(_DSET) // 2
assert len(_DSET) % 2 == 0 and 0 not in _DSET


def _ap3(base2d, d1, n1, d2, n2):
    return bass.AP(
        tensor=base2d.tensor,
        offset=base2d.offset,
        ap=[list(base2d.ap[0]), [d1, n1], [d2, n2]],
    )


def _skip_ones_ap(base2d):
    """[P, 2, 32] AP over cols {1..32, 34..65} of a [P, >=66] slice starting
    at col 1 — the V-block minus the two ones columns."""
    return _ap3(base2d, 33, 2, 1, 32)


def _build():
    nc = bacc.Bacc(None, target_bir_lowering=False)
    xt_d = nc.declare_dram_parameter("xt", [C, N], BF16, isOutput=False)
    wqkT_d = nc.declare_dram_parameter("wqkT", [C, 512], BF16, isOutput=False)
    wvT_d = nc.declare_dram_parameter("wvT", [C, VBLK], BF16, isOutput=False)
    woT_d = nc.declare_dram_parameter("woT", [C, 256], BF16, isOutput=False)
    out_d = nc.declare_dram_parameter("out", [C, N], F32, isOutput=True)
    recd = nc.dram_tensor("recd", [1, 1024], F32)  # reciprocal bounce for bcast

    EXP = mybir.ActivationFunctionType.Exp
    IDENT = mybir.ActivationFunctionType.Identity

    with tile.TileContext(nc) as tc:
        with (
            tc.tile_pool(name="singles", bufs=1) as singles,
            tc.tile_pool(name="pts", bufs=PTBUFS) as pts,
            tc.tile_pool(name="ptps", bufs=PTPBUFS) as ptps,
            tc.tile_pool(name="simpool", bufs=3, space="PSUM") as simpool,
            tc.tile_pool(name="avpool", bufs=1, space="PSUM") as avpool,
        ):
            xt_s = singles.tile([C, N], BF16)
            wqkT_s = singles.tile([C, 512], BF16)
            wvT_s = singles.tile([C, VBLK], BF16)
            woT_s = singles.tile([C, 256], BF16)
            qt0 = singles.tile([C, N], BF16)  # head0 Q: rows 0-31, rest zero
            kt0 = singles.tile([C, N], BF16)  # head0 K
            qt1 = singles.tile([C, N], BF16)  # head1 Q
            kt1 = singles.tile([C, N], BF16)  # head1 K
            # one vt tile per j-strip: [1|Vh0|1|Vh1|zeros] padded to 161 cols
            # so both heads' 128-wide lhsT APs stay in untiled (128,128) mode.
            vts = [
                singles.tile([C, 161], BF16, tag=f"vt{j}", name=f"vt{j}")
                for j in range(NJS)
            ]
            # fp8 paired V-blocks for DoubleRow: per (pair, head) a [128, 128]
            # tile, halves = [1|V(32)|0(31)] of the pair's two strips.
            vtp = [
                [
                    singles.tile([C, 128], FP8, tag=f"vtp{p}h{h}", name=f"vtp{p}h{h}")
                    for h in range(2)
                ]
                for p in range(NPAIR)
            ]
            onesb = singles.tile([C, 1], BF16)  # colsum rhs
            corrS = singles.tile([C, 2], F32)  # per-head expm1 correction col
            # per-group scratch: [a-chunk | b-chunk] halves, reused group to
            # group (WAR deps keep it safe; groups are far apart in time)
            avu = singles.tile([DH + 1, 1024], F32)
            avn = singles.tile([C, 2 * N], BF16)  # rows 33-127 zero
            rec = singles.tile([1, 1024], F32)  # 1/denom
            bc = singles.tile([DH + 1, 1024], F32)  # broadcast recips
            outs = singles.tile([C, 1024], F32)  # po evacuation

            nc.sync.dma_start(out=wqkT_s[:], in_=wqkT_d[:])
            for icd in range(NICH):
                nc.sync.dma_start(
                    out=xt_s[:, IC * icd : IC * (icd + 1)],
                    in_=xt_d[:, IC * icd : IC * (icd + 1)],
                )
            nc.gpsimd.dma_start(out=wvT_s[:], in_=wvT_d[:])
            nc.gpsimd.dma_start(out=woT_s[:], in_=woT_d[:])
            # GPSIMD prologue: only avn's junk rows (they feed the final
            # projection's contraction and must not be NaN; rows 0-32 are
            # fully written by the normalize muls) and the colsum rhs. The
            # ones columns of vts/vtp trickle on GPSIMD during early slots;
            # no zero-fills anywhere else: all junk regions (vts/vtp pad
            # columns) only ever produce PSUM rows that are never read.
            # engine APs starting at partition>0 may touch at most 32
            # partitions; zero avn's junk rows in 32-partition chunks
            # (row 32 overlaps the mul-written region: harmless WAW edge).
            nc.gpsimd.memset(avn[32:64, :], 0.0)
            nc.gpsimd.memset(avn[64:96, :], 0.0)
            nc.gpsimd.memset(avn[96:128, :], 0.0)
            nc.gpsimd.memset(onesb[:], 1.0)

            qts = [qt0, qt1]
            kts = [kt0, kt1]

            def qk_proj_one(h, icx, copy_q, copy_k):
                ps = simpool.tile([128, 1024], F32, tag="sim")
                for half in range(2):  # 0: Q, 1: K
                    cc = 2 * half + h
                    nc.tensor.matmul(
                        ps[:, IC * half : IC * (half + 1)],
                        lhsT=wqkT_s[:, 128 * cc : 128 * (cc + 1)],
                        rhs=xt_s[:, IC * icx : IC * (icx + 1)],
                        start=True,
                        stop=True,
                    )
                for half, eng, dst in ((0, copy_q, qts[h]), (1, copy_k, kts[h])):
                    eng(
                        dst[:, IC * icx : IC * (icx + 1)],
                        ps[:, IC * half : IC * (half + 1)],
                    )

            # q-copies on ACT, k-copies on DVE: halves the prologue copy
            # serialization so the first sims start ~5us sooner
            for icd in range(NICH):
                qk_proj_one(0, icd, nc.scalar.copy, nc.vector.tensor_copy)

            def v_proj_one(jc):
                pv = simpool.tile([128, 1024], F32, tag="sim")
                nc.tensor.matmul(
                    pv[:, 0:VBLK],
                    lhsT=xt_s[:, JS * jc : JS * (jc + 1)],
                    rhs=wvT_s[:],
                    start=True,
                    stop=True,
                )
                nc.scalar.copy(
                    _skip_ones_ap(vts[jc][:, 1:2]), _skip_ones_ap(pv[:, 1:2])
                )
                # fp8 paired copies for DoubleRow strips (bf16 SBUF -> fp8,
                # on GPSIMD so neither exp engine pays for it)
                if jc in _DSET:
                    k = _DSET.index(jc)
                    p, halfi = k // 2, k % 2
                    for h in range(2):
                        nc.gpsimd.tensor_copy(
                            vtp[p][h][:, 64 * halfi + 1 : 64 * halfi + 33],
                            vts[jc][:, 33 * h + 1 : 33 * h + 33],
                        )

            VLEAD = 6
            for jc in range(VLEAD):
                v_proj_one(jc)

            groups = [(h, ip) for h in range(2) for ip in range(4)]
            gstate = [dict(ptp={}, npop=0) for _ in groups]
            av_q = []  # ('b', pt, js, gidx) | ('dr', ptp_tile, pair, gidx)
            po_q = []  # (due_slot, gidx)
            ENTRIES = (NJS - len(_DSET)) + NPAIR  # av_q entries per group

            def emit_corr():
                # corr[m] = sum over DR strips of colsum of the EXACT (bf16)
                # V-block, per head. Using exact V here (not the fp8 copy the
                # DR matmuls see) makes the fp8 V quantization error multiply
                # expm1 (~0.1) instead of attn (~1.0) — a 10x suppression.
                # One 1-row matmul per (DR strip, head), stationary = the same
                # padded vts block the bf16 AVs use, so tile mode is (128,128).
                cp = simpool.tile([128, 1024], F32, tag="sim", name="corrP")
                for h in range(2):
                    for k, jc in enumerate(_DSET):
                        nc.tensor.matmul(
                            cp[:, h : h + 1],
                            lhsT=vts[jc][:, 33 * h : 33 * h + 128],
                            rhs=onesb[:],
                            start=(k == 0),
                            stop=(k == 2 * NPAIR - 1),
                            skip_group_check=True,
                        )
                nc.vector.tensor_copy(corrS[:], cp[:, 0:2])

            def emit_av(entry):
                kind, gidx = entry[0], entry[-1]
                st = gstate[gidx]
                h, ip = groups[gidx]
                if st["npop"] == 0:
                    st["ava"] = avpool.tile(
                        [C, IC], F32, tag="av_a", name=f"ava{gidx}"
                    )
                    st["avb"] = avpool.tile(
                        [C, IC], F32, tag="av_b", name=f"avb{gidx}"
                    )
                st["npop"] += 1
                first = st["npop"] == 1
                last = st["npop"] == ENTRIES
                if kind == "b":
                    _, apt, ajs, _ = entry
                    for t, av in enumerate((st["ava"], st["avb"])):
                        nc.tensor.matmul(
                            av[:],
                            lhsT=vts[ajs][:, 33 * h : 33 * h + 128],
                            rhs=apt[:, IC * t : IC * (t + 1)],
                            start=first,
                            stop=last,
                            skip_group_check=True,
                        )
                else:
                    _, ptile, pair, _ = entry
                    lhs3 = _ap3(vtp[pair][h][:, 0:1], 64, 2, 1, 64)
                    for t, av in enumerate((st["ava"], st["avb"])):
                        nc.tensor.matmul(
                            av[0:64, :],
                            lhsT=lhs3,
                            rhs=_ap3(ptile[:, IC * t : IC * t + 1], 1024, 2, 1, 512),
                            start=False,
                            stop=last,
                            perf_mode=DRMODE,
                            skip_group_check=True,
                        )
                return last

            def emit_epilogue(gidx, slot):
                h, ip = groups[gidx]
                st = gstate[gidx]
                hoff = N * h
                for t, av in enumerate((st["ava"], st["avb"])):
                    half = slice(512 * t, 512 * (t + 1))
                    # evacuate + add the expm1 correction column as bias;
                    # a-half on ACT, b-half on DVE
                    if t == 0:
                        nc.scalar.activation(
                            avu[:, half],
                            av[0 : DH + 1, :],
                            IDENT,
                            bias=corrS[0 : DH + 1, h : h + 1],
                        )
                    else:
                        nc.vector.tensor_scalar_add(
                            avu[:, half],
                            av[0 : DH + 1, :],
                            corrS[0 : DH + 1, h : h + 1],
                        )
                    nc.vector.reciprocal_approx_fast(
                        rec[0:1, half], avu[0:1, half]
                    )
                # one bounce round-trip for both chunks (fewer serial DMAs)
                nc.sync.dma_start(out=recd[0:1, 0:1024], in_=rec[0:1, 0:1024])
                dsl = recd[0:1, 0:1024]
                nc.sync.dma_start(
                    out=bc[:, 0:1024],
                    in_=bass.AP(
                        tensor=dsl.tensor,
                        offset=dsl.offset,
                        ap=[[0, DH + 1]] + list(dsl.ap[1:]),
                    ),
                )
                # normalize muls in parallel: a on DVE, b on GPSIMD
                for t in range(2):
                    ic = 2 * ip + t
                    sl = slice(hoff + IC * ic, hoff + IC * (ic + 1))
                    half = slice(512 * t, 512 * (t + 1))
                    eng = nc.vector if t == 0 else nc.gpsimd
                    eng.tensor_mul(
                        avn[0 : DH + 1, sl], avu[:, half], bc[:, half]
                    )
                if h == 1:
                    po_q.append((slot + PO_DELAY, gidx))

            def emit_po(gidx):
                h, ip = groups[gidx]
                po = simpool.tile([128, 1024], F32, tag="sim", name=f"po{gidx}")
                for t in range(2):
                    ic = 2 * ip + t
                    nc.tensor.matmul(
                        po[:, 512 * t : 512 * t + IC],
                        lhsT=woT_s[:, 0:128],
                        rhs=avn[:, IC * ic : IC * (ic + 1)],
                        start=True,
                        stop=False,
                    )
                    nc.tensor.matmul(
                        po[:, 512 * t : 512 * t + IC],
                        lhsT=woT_s[:, 128:256],
                        rhs=avn[:, N + IC * ic : N + IC * (ic + 1)],
                        start=False,
                        stop=True,
                    )
                # bias is folded into the projection (avn row 0 = denom*recip
                # = 1, woT row 0 of block 0 is bo); evacuate the two chunks on
                # different engines so they run in parallel
                for t in range(2):
                    ic = 2 * ip + t
                    half = slice(512 * t, 512 * (t + 1))
                    eng = nc.scalar.copy if t == 0 else nc.vector.tensor_copy
                    eng(outs[:, half], po[:, half])
                    nc.sync.dma_start(
                        out=out_d[:, IC * ic : IC * (ic + 1)],
                        in_=outs[:, half],
                    )

            slot = 0
            for gidx, (h, ip) in enumerate(groups):
                qt, kt = qts[h], kts[h]
                ica, icb = 2 * ip, 2 * ip + 1
                for js in range(NJS):
                    while po_q and po_q[0][0] <= slot:
                        emit_po(po_q.pop(0)[1])
                    if gidx == 0:
                        nc.gpsimd.memset(
                            _ap3(vts[js][:, 0:1], 33, 2, 1, 1), 1.0
                        )
                        if js < NPAIR:
                            for hh in range(2):
                                nc.gpsimd.memset(
                                    _ap3(vtp[js][hh][:, 0:1], 64, 2, 1, 1), 1.0
                                )
                    if h == 0 and ip == 0 and js < NJS - VLEAD:
                        v_proj_one(js + VLEAD)
                    if h == 0 and ip == 1 and js == 0 and NPAIR > 0:
                        emit_corr()
                    if h == 0 and ip in (1, 2) and js % 8 == 0:
                        qk_proj_one(
                            1,
                            4 * (ip - 1) + js // 8,
                            nc.scalar.copy,
                            nc.vector.tensor_copy,
                        )
                    sim = simpool.tile([128, 1024], F32, tag="sim")
                    for t, icx in enumerate((ica, icb)):
                        nc.tensor.matmul(
                            sim[:, IC * t : IC * (t + 1)],
                            lhsT=kt[:, JS * js : JS * (js + 1)],
                            rhs=qt[:, IC * icx : IC * (icx + 1)],
                            start=True,
                            stop=True,
                        )
                    if js in _DSET:
                        k = _DSET.index(js)
                        pair, halfi = k // 2, k % 2
                        st = gstate[gidx]
                        if halfi == 0:
                            st["ptp"][pair] = ptps.tile(
                                [128, 2048], FP8, tag="ptp",
                                name=f"ptp{gidx}_{pair}",
                            )
                        ptile = st["ptp"][pair]
                        dst = ptile[:, 1024 * halfi : 1024 * (halfi + 1)]
                        if _use_dve(js):
                            nc.vector._custom_dve(
                                _EXPM1_OP, out=dst, in0=sim[:],
                                s0=_EXPC2, s1=_EXPC3, imm2=_EXPC4,
                            )
                        else:
                            # ACT exp in bf16, GPSIMD turns it into fp8 expm1
                            pt = pts.tile([128, 1024], BF16, tag="pt")
                            nc.scalar.activation(pt[:], sim[:], EXP)
                            nc.gpsimd.tensor_scalar_add(dst, pt[:], -1.0)
                        if halfi == 1:
                            av_q.append(("dr", ptile, pair, gidx))
                    else:
                        pt = pts.tile([128, 1024], BF16, tag="pt")
                        nc.scalar.activation(pt[:], sim[:], EXP)
                        av_q.append(("b", pt, js, gidx))
                    if len(av_q) > AVLAG:
                        entry = av_q.pop(0)
                        if emit_av(entry):
                            emit_epilogue(entry[-1], slot)
                    slot += 1

            while av_q:
                entry = av_q.pop(0)
                if emit_av(entry):
                    emit_epilogue(entry[-1], slot)
                slot += 1
            while po_q:
                emit_po(po_q.pop(0)[1])
    nc.finalize()
    return nc


_nc_cache = None


def _get_nc():
    global _nc_cache
    if _nc_cache is None:
        _nc_cache = _build()
    return _nc_cache


def make_in_maps(x, wq, wk, wv, wo, bo):
    b = 4
    xt = np.asarray(x, np.float32).reshape(b, C, N)
    wq = np.asarray(wq, np.float32)
    wk = np.asarray(wk, np.float32)
    wv = np.asarray(wv, np.float32)
    wo = np.asarray(wo, np.float32)
    bo = np.asarray(bo, np.float32)
    scale = DH ** (-0.5)

    def bf(a):
        return np.ascontiguousarray(a.astype(ml_dtypes.bfloat16))

    in_maps = []
    for core in range(8):
        bi, hp = core // 2, core % 2
        wq2 = wq[64 * hp : 64 * hp + 64] * scale
        wk2 = wk[64 * hp : 64 * hp + 64]
        wv2 = wv[64 * hp : 64 * hp + 64]
        wqkT = np.zeros((C, 512), np.float32)
        wqkT[:, 0:32] = wq2.T[:, 0:32]  # Qh0
        wqkT[:, 128:160] = wq2.T[:, 32:64]  # Qh1
        wqkT[:, 256:288] = wk2.T[:, 0:32]  # Kh0
        wqkT[:, 384:416] = wk2.T[:, 32:64]  # Kh1
        wvT = np.zeros((C, VBLK), np.float32)  # cols 0,33 stay 0 (ones in SBUF)
        wvT[:, 1:33] = wv2.T[:, 0:32]
        wvT[:, 34:66] = wv2.T[:, 32:64]
        woT = np.zeros((C, 256), np.float32)
        woT[1:33, 0:128] = wo[:, 64 * hp : 64 * hp + 32].T
        woT[1:33, 128:256] = wo[:, 64 * hp + 32 : 64 * hp + 64].T
        if hp == 0:
            woT[0, 0:128] = bo  # bias rides avn row 0 (= denom/denom = 1)
        in_maps.append(
            {
                "xt": bf(xt[bi]),
                "wqkT": bf(wqkT),
                "wvT": bf(wvT),
                "woT": bf(woT),
            }
        )
    return in_maps


def kernel(x, wq, wk, wv, wo, bo):
    global _last_results
    in_maps = make_in_maps(x, wq, wk, wv, wo, bo)
    nc = _get_nc()
    res = run_bass_kernel_spmd(nc, in_maps, core_ids=list(range(8)))
    _last_results = res
    outs = res.results
    out = np.zeros((4, C, N), np.float32)
    for bi in range(4):
        out[bi] = np.asarray(outs[2 * bi]["out"], np.float32) + np.asarray(
            outs[2 * bi + 1]["out"], np.float32
        )
    return out.reshape(4, C, 64, 64)
